# revision 5
# baseline (speedup 1.0000x reference)
"""DIEN kernel for Trainium2 (Bass/Tile), 8-way data-parallel over batch.

Layout: transposed activations [feature (<=128 partitions), batch (free dim)].
Per core: 512 batch rows, T=50 steps. GRU / attention / AUGRU fused in one
skewed loop (ATT 1 step behind GRU, AUGRU SKEW_AU behind), head at the end.

v2 design (mixed precision, engine-balanced):
- x-side projections run as fp8e4 DoubleRow matmuls (0.5 cyc/row): the host
  quantizes hist to fp8, and each gate's bias rides the DR pair's second slot
  against a static ones vector (stationary row-0 = bias row), so gate psums
  arrive bias-included with zero extra ops.
- candidate folds: t1 = (phh+b1h)*r is an STT (no DVE fast mode either way)
  written directly as fp8 into the moving pair next to x8; one DR (I | Wh)
  then computes xh + t1 in a single 107ns matmul.
- recurrent state h/h2/hs stays bf16 so every combine TT gets the DVE 2x
  mode; h-side matmuls are plain bf16 (1 cyc/row, same as f32r).
- attention: relu1/relu2 emit fp8 directly, pa2/pa3 are DR-padded; the
  abc@news term is precomputed on host (hi+lo fp8 pair, folded via DR(I|I)).
- ats broadcast via gpsimd partition_broadcast (no PSUM bank, no PE).
- PSUM banks (8): zr[2] ur[2] cand[1] hside[1] pa1[1] pa2+a3 shared[1].
"""
import sys

sys.path.insert(0, "/opt/trn_rl_repo")

import numpy as np
import ml_dtypes

import concourse.bass as bass
import concourse.mybir as mybir
import concourse.tile as tile
from concourse import bacc
from concourse.bass_utils import run_bass_kernel_spmd

B, T, D, U = 4096, 50, 128, 128
NCORES = 8
BL = B // NCORES  # 512
P = 128
F32 = mybir.dt.float32
F32R = mybir.dt.float32r
BF16 = mybir.dt.bfloat16
F8 = mybir.dt.float8e4
NP_F8 = ml_dtypes.float8_e4m3
NP_BF = ml_dtypes.bfloat16
AF = mybir.ActivationFunctionType
OP = mybir.AluOpType
DR = mybir.MatmulPerfMode.DoubleRow
LEAKY = 0.0003

CFG = {
    "skew_au": 6,
    # engine per op: "v" = vector(DVE), "g" = gpsimd(Pool)
    "eng": {
        "t1": "v", "t2": "g", "mmj": "v",
        "d": "v", "w": "v", "hn": "v",
        "u_": "v", "d2": "v", "m2": "v", "hn2": "v",
        "relu1": "g", "relu2": "g", "patb": "g",
    },
    "debug": False,
}

# bias column indices in the packed [128, 16] bias tensor
B1H, B0H, BU, BR2, BC, B1, B2, B3, DB1A, DB1B, DB2, FB = range(12)


def _eng(nc, key):
    return nc.vector if CFG["eng"][key] == "v" else nc.gpsimd


def build_nc(debug=False):
    nc = bacc.Bacc()
    SKEW_AU = CFG["skew_au"]
    NITER = T + SKEW_AU

    # ---------------- DRAM inputs
    hist8 = nc.dram_tensor("hist8", [T, P, BL], F8, kind="ExternalInput")
    news_b = nc.dram_tensor("news_b", [P, BL], BF16, kind="ExternalInput")
    news_r = nc.dram_tensor("news_r", [P, BL], F32R, kind="ExternalInput")
    # fp8 DR stationary pairs [128, 2, 128]: (A on slot0, B on slot1)
    w_z8 = nc.dram_tensor("w_z8", [P, 2, P], F8, kind="ExternalInput")  # (Wz | e0*bz)
    w_r8 = nc.dram_tensor("w_r8", [P, 2, P], F8, kind="ExternalInput")  # (Wr | e0*br)
    w_c8 = nc.dram_tensor("w_c8", [P, 2, P], F8, kind="ExternalInput")  # (I  | Wh)
    w_vc8 = nc.dram_tensor("w_vc8", [P, 2, P], F8, kind="ExternalInput")  # (I | 0) AUGRU fold
    w_a28 = nc.dram_tensor("w_a28", [P, 2, 64], F8, kind="ExternalInput")  # (a2w | 0)
    w_an8 = nc.dram_tensor("w_an8", [P, 2, P], F8, kind="ExternalInput")  # (I | I) anews fold
    w_a38 = nc.dram_tensor("w_a38", [64, 8, 4], F8, kind="ExternalInput")  # 4x (w3col | 0)
    # bf16 weights (h-side + attention + AUGRU x-side)
    w_gu = nc.dram_tensor("w_gu", [P, 3 * U], BF16, kind="ExternalInput")
    w_aw = nc.dram_tensor("w_aw", [P, 3 * U], BF16, kind="ExternalInput")
    w_au = nc.dram_tensor("w_au", [P, 3 * U], BF16, kind="ExternalInput")
    w_ax = nc.dram_tensor("w_ax", [P, P], BF16, kind="ExternalInput")
    w_am = nc.dram_tensor("w_am", [P, P], BF16, kind="ExternalInput")
    anews8 = nc.dram_tensor("anews8", [P, 2, BL], F8, kind="ExternalInput")  # (hi | lo)
    # head (f32r as baseline)
    w_d1a = nc.dram_tensor("w_d1a", [P, 256], F32R, kind="ExternalInput")
    w_d1b = nc.dram_tensor("w_d1b", [P, 256], F32R, kind="ExternalInput")
    w_d2a = nc.dram_tensor("w_d2a", [P, P], F32R, kind="ExternalInput")
    w_d2b = nc.dram_tensor("w_d2b", [P, P], F32R, kind="ExternalInput")
    w_f = nc.dram_tensor("w_f", [P, 1], F32R, kind="ExternalInput")
    biases = nc.dram_tensor("biases", [P, 16], F32, kind="ExternalInput")
    y_out = nc.dram_tensor("y", [1, BL], F32, kind="ExternalOutput")
    if debug:
        hg_out = nc.dram_tensor("hg", [P, BL], F32, kind="ExternalOutput")
        h2_out = nc.dram_tensor("h2f", [P, BL], F32, kind="ExternalOutput")

    with tile.TileContext(nc) as tc:
        import contextlib

        ctx = contextlib.ExitStack()
        with ctx:
            wp = ctx.enter_context(tc.tile_pool(name="wp", bufs=1))
            ps = ctx.enter_context(tc.tile_pool(name="ps", bufs=1, space="PSUM"))

            # ---------------- load weights
            def wtile(name, dram, shape, dt):
                t = wp.tile(shape, dt, name=name, tag=name)
                nc.sync.dma_start(t[:], dram[:])
                return t

            z8 = wtile("z8", w_z8, [P, 2, P], F8)
            r8 = wtile("r8", w_r8, [P, 2, P], F8)
            c8 = wtile("c8", w_c8, [P, 2, P], F8)
            vc8 = wtile("vc8", w_vc8, [P, 2, P], F8)
            a28 = wtile("a28", w_a28, [P, 2, 64], F8)
            an8 = wtile("an8", w_an8, [P, 2, P], F8)
            a38 = wtile("a38", w_a38, [64, 8, 4], F8)
            gu = wtile("gu", w_gu, [P, 3 * U], BF16)
            aw = wtile("aw", w_aw, [P, 3 * U], BF16)
            au = wtile("au", w_au, [P, 3 * U], BF16)
            ax = wtile("ax", w_ax, [P, P], BF16)
            am = wtile("am", w_am, [P, P], BF16)
            anp = wtile("anp", anews8, [P, 2, BL], F8)
            d1a = wtile("d1a", w_d1a, [P, 256], F32R)
            d1b = wtile("d1b", w_d1b, [P, 256], F32R)
            d2a = wtile("d2a", w_d2a, [P, P], F32R)
            d2b = wtile("d2b", w_d2b, [P, P], F32R)
            fw = wtile("fw", w_f, [P, 1], F32R)
            bia = wtile("bia", biases, [P, 16], F32)
            newsb = wtile("newsb", news_b, [P, BL], BF16)
            newsr = wtile("newsr", news_r, [P, BL], F32R)

            def bap(col, rows=P):
                return bia[0:rows, col : col + 1]

            # ---------------- static SBUF rings (manual)
            NG = 3
            grings = []  # [t18 | x8 | ones8] fp8
            for k in range(NG):
                g = wp.tile([P, 3, BL], F8, name=f"gring{k}", tag=f"gring{k}")
                nc.vector.memset(g[:, 2, :], 1.0)
                grings.append(g)
            vrings = []  # [t28 | zeros] fp8
            for k in range(2):
                v = wp.tile([P, 2, BL], F8, name=f"vring{k}", tag=f"vring{k}")
                nc.vector.memset(v[:, 1, :], 0.0)
                vrings.append(v)
            a1rings = []  # [a1 | zeros] fp8
            for k in range(2):
                a1r = wp.tile([P, 2, BL], F8, name=f"a1ring{k}", tag=f"a1ring{k}")
                nc.vector.memset(a1r[:, 1, :], 0.0)
                a1rings.append(a1r)
            a2rings = []  # [a2 | zeros] fp8 (64 partitions)
            for k in range(2):
                a2r = wp.tile([64, 2, BL], F8, name=f"a2ring{k}", tag=f"a2ring{k}")
                nc.vector.memset(a2r[:, 1, :], 0.0)
                a2rings.append(a2r)

            NH = 8
            hs_ring = []  # GRU outputs bf16
            for k in range(NH):
                h = wp.tile([P, BL], BF16, name=f"hs{k}", tag=f"hs{k}")
                hs_ring.append(h)
            h2_ring = []
            for k in range(2):
                h2 = wp.tile([P, BL], BF16, name=f"h2_{k}", tag=f"h2_{k}")
                h2_ring.append(h2)
            h_init = wp.tile([P, BL], BF16, name="h_init", tag="h_init")
            nc.vector.memset(h_init[:], 0.0)
            h2_init = wp.tile([P, BL], BF16, name="h2_init", tag="h2_init")
            nc.vector.memset(h2_init[:], 0.0)

            # per-iter small tiles via pools (auto ring by tag)
            sp = ctx.enter_context(tc.tile_pool(name="sp", bufs=2))

            # PSUM banks (8): zr[2] (GRU z|r, then AUGRU u|r2 time-muxed),
            # cand[2] (pxh/pxc alternate), hside[2] (phh/prc alternate),
            # pa1[1], pa2 shares with a3? no: pa2[?]... a3[1]
            def ps_tile(name, shape, tag, bufs=1):
                return ps.tile(shape, F32, name=name, tag=tag, bufs=bufs)

            mm = nc.tensor.matmul

            hs_at = lambda t: h_init if t < 0 else hs_ring[t % NH]
            h2_at = lambda s: h2_init if s < 0 else h2_ring[s % 2]

            ats_tiles = {}
            a3_psums = {}

            # prefetch x8 for t=0
            nc.sync.dma_start(grings[0][:, 1, :], hist8[0])

            for i in range(NITER):
                t = i if i < T else None
                j = i - 1 if 0 <= i - 1 < T else None
                s = i - SKEW_AU if 0 <= i - SKEW_AU < T else None

                # ---------------- GRU step t
                if t is not None:
                    G = grings[t % NG]
                    if t + 1 < T:
                        nc.sync.dma_start(
                            grings[(t + 1) % NG][:, 1, :], hist8[t + 1]
                        )
                    h_prev = hs_at(t - 1)

                    pzr = ps_tile(f"pzr{t}", [P, 2 * BL], "zr")
                    # z: DR(Wz|bz-rider) on (x8, ones8) + bf16 Uz@h
                    mm(pzr[:, 0:BL], z8[:], G[:, 1:3, :], start=True, stop=False,
                       perf_mode=DR)
                    mm(pzr[:, 0:BL], gu[:, 0:U], h_prev[:], start=False, stop=True)
                    mm(pzr[:, BL : 2 * BL], r8[:], G[:, 1:3, :], start=True,
                       stop=False, perf_mode=DR)
                    mm(pzr[:, BL : 2 * BL], gu[:, U : 2 * U], h_prev[:],
                       start=False, stop=True)
                    zr = sp.tile([P, 2 * BL], BF16, name=f"zr{t}", tag="zr_sb")
                    nc.scalar.activation(zr[:], pzr[:], AF.Sigmoid)
                    z, r = zr[:, 0:BL], zr[:, BL : 2 * BL]

                    phh = ps_tile(f"phh{t}", [P, BL], "hside", bufs=2)
                    mm(phh[:], gu[:, 2 * U : 3 * U], h_prev[:], start=True, stop=True)
                    # t18 = (phh + b1h) * r -> fp8 into G slot 0
                    _eng(nc, "t1").scalar_tensor_tensor(
                        G[:, 0, :], phh[:], bap(B1H), r, OP.add, OP.mult
                    )
                    pxh = ps_tile(f"pxh{t}", [P, BL], "cand", bufs=2)
                    mm(pxh[:], c8[:], G[:, 0:2, :], start=True, stop=True,
                       perf_mode=DR)
                    hc = sp.tile([P, BL], BF16, name=f"hc{t}", tag="hc")
                    nc.scalar.activation(hc[:], pxh[:], AF.Tanh, bias=bap(B0H))

                    # hn = hc + z*(h - hc)
                    d = sp.tile([P, BL], BF16, name=f"d{t}", tag="d")
                    _eng(nc, "d").tensor_sub(d[:], h_prev[:], hc[:])
                    w_ = sp.tile([P, BL], BF16, name=f"w{t}", tag="w")
                    _eng(nc, "w").tensor_mul(w_[:], z, d[:])
                    hn = hs_ring[t % NH]
                    _eng(nc, "hn").tensor_add(hn[:], hc[:], w_[:])

                # ---------------- attention step j
                if j is not None:
                    hs_j = hs_at(j)
                    mmj = sp.tile([P, BL], BF16, name=f"mmj{j}", tag="mmj")
                    _eng(nc, "mmj").tensor_mul(mmj[:], hs_j[:], newsb[:])
                    pa1 = ps_tile(f"pa1{j}", [P, BL], "pa1")
                    mm(pa1[:], an8[:], anp[:], start=True, stop=False, perf_mode=DR)
                    mm(pa1[:], ax[:], hs_j[:], start=False, stop=False)
                    mm(pa1[:], am[:], mmj[:], start=False, stop=True)
                    A1 = a1rings[j % 2]
                    _eng(nc, "relu1").tensor_scalar(
                        A1[:, 0, :], pa1[:], bap(B1), 0.0, OP.add, OP.max
                    )
                    pa2 = ps_tile(f"pa2{j}", [64, BL], "pa1")
                    mm(pa2[:], a28[:], A1[:, 0:2, :], start=True, stop=True,
                       perf_mode=DR)
                    A2 = a2rings[j % 2]
                    _eng(nc, "relu2").tensor_scalar(
                        A2[:, 0, :], pa2[:], bap(B2, rows=64), 0.0, OP.add, OP.max
                    )
                    g = j // 4
                    k4 = j % 4
                    if k4 == 0:
                        a3_psums[g] = ps.tile([4, BL], F32, name=f"pa3{g}", tag="a3")
                    mm(a3_psums[g][:], a38[:, 2 * k4 : 2 * k4 + 2, :],
                       A2[:, 0:2, :], start=(k4 == 0),
                       stop=(k4 == 3 or j == T - 1), perf_mode=DR)
                    if k4 == 3 or j == T - 1:
                        k = k4 + 1
                        ats = sp.tile([4, BL], BF16, name=f"ats{g}", tag="ats")
                        nc.scalar.activation(
                            ats[0:k, :], a3_psums[g][0:k, :], AF.Sigmoid,
                            bias=bap(B3, rows=k),
                        )
                        ats_tiles[g] = ats

                # ---------------- AUGRU step s
                if s is not None:
                    V = vrings[s % 2]
                    h2_prev = h2_at(s - 1)
                    hs_s = hs_at(s)

                    pat = sp.tile([P, BL], BF16, name=f"pat{s}", tag="pat")
                    nc.gpsimd.partition_broadcast(
                        pat[:], ats_tiles[s // 4][s % 4 : s % 4 + 1, :]
                    )

                    pur = ps_tile(f"pur{s}", [P, 2 * BL], "zr")
                    mm(pur[:, 0:BL], aw[:, 0:U], hs_s[:], start=True, stop=False)
                    mm(pur[:, 0:BL], au[:, 0:U], h2_prev[:], start=False, stop=True)
                    mm(pur[:, BL : 2 * BL], aw[:, U : 2 * U], hs_s[:],
                       start=True, stop=False)
                    mm(pur[:, BL : 2 * BL], au[:, U : 2 * U], h2_prev[:],
                       start=False, stop=True)
                    ut = sp.tile([P, BL], BF16, name=f"u{s}", tag="u_sb")
                    nc.scalar.activation(ut[:], pur[:, 0:BL], AF.Sigmoid,
                                         bias=bap(BU))
                    r2t = sp.tile([P, BL], BF16, name=f"r2{s}", tag="r2_sb")
                    nc.scalar.activation(r2t[:], pur[:, BL : 2 * BL], AF.Sigmoid,
                                         bias=bap(BR2))

                    prc = ps_tile(f"prc{s}", [P, BL], "hside", bufs=2)
                    mm(prc[:], au[:, 2 * U : 3 * U], h2_prev[:], start=True,
                       stop=True)
                    # t28 = prc * r2 -> fp8 into V slot 0
                    _eng(nc, "t2").scalar_tensor_tensor(
                        V[:, 0, :], prc[:], 0.0, r2t[:], OP.add, OP.mult
                    )
                    pxc = ps_tile(f"pxc{s}", [P, BL], "cand", bufs=2)
                    mm(pxc[:], aw[:, 2 * U : 3 * U], hs_s[:], start=True,
                       stop=False)
                    mm(pxc[:], vc8[:], V[:, 0:2, :], start=False, stop=True,
                       perf_mode=DR)
                    c = sp.tile([P, BL], BF16, name=f"c{s}", tag="c")
                    nc.scalar.activation(c[:], pxc[:], AF.Tanh, bias=bap(BC))

                    # hn2 = h2 + u_*(c - h2), u_ = pat*u
                    u_ = sp.tile([P, BL], BF16, name=f"u_{s}", tag="u_")
                    _eng(nc, "u_").tensor_mul(u_[:], pat[:], ut[:])
                    d2 = sp.tile([P, BL], BF16, name=f"d2{s}", tag="d2")
                    _eng(nc, "d2").tensor_sub(d2[:], c[:], h2_prev[:])
                    m2 = sp.tile([P, BL], BF16, name=f"m2{s}", tag="m2")
                    _eng(nc, "m2").tensor_mul(m2[:], u_[:], d2[:])
                    hn2 = h2_ring[s % 2]
                    _eng(nc, "hn2").tensor_add(hn2[:], h2_prev[:], m2[:])

            # ---------------- deep head (one-time, f32r)
            h2f = h2_ring[(T - 1) % 2]
            if debug:
                hgf = sp.tile([P, BL], F32, name="hgf", tag="hgf")
                nc.scalar.copy(hgf[:], hs_ring[(T - 1) % NH][:])
                nc.sync.dma_start(hg_out[:], hgf[:])
                h2c = sp.tile([P, BL], F32, name="h2c", tag="h2c")
                nc.scalar.copy(h2c[:], h2f[:])
                nc.sync.dma_start(h2_out[:], h2c[:])

            o1 = sp.tile([P, 2 * BL], F32R, name="o1", tag="o1")
            for mch in range(2):
                po = ps_tile(f"po1_{mch}", [P, BL], "cand", bufs=2)
                mm(po[:], d1a[:, mch * P : (mch + 1) * P], h2f[:], start=True,
                   stop=False)
                mm(po[:], d1b[:, mch * P : (mch + 1) * P], newsr[:], start=False,
                   stop=True)
                nc.scalar.activation(
                    o1[:, mch * BL : (mch + 1) * BL], po[:], AF.Lrelu,
                    bias=bap(DB1A + mch), alpha=LEAKY,
                )
            po2 = ps_tile("po2", [P, BL], "hside", bufs=2)
            mm(po2[:], d2a[:], o1[:, 0:BL], start=True, stop=False)
            mm(po2[:], d2b[:], o1[:, BL : 2 * BL], start=False, stop=True)
            o2 = sp.tile([P, BL], F32R, name="o2", tag="o2")
            nc.scalar.activation(o2[:], po2[:], AF.Lrelu, bias=bap(DB2),
                                 alpha=LEAKY)
            py = ps_tile("py", [1, BL], "pa1")
            mm(py[:], fw[:], o2[:], start=True, stop=True)
            y_sb = sp.tile([1, BL], F32, name="y_sb", tag="ysb")
            nc.scalar.activation(y_sb[:], py[:], AF.Sigmoid, bias=bap(FB, rows=1))
            nc.sync.dma_start(y_out[:], y_sb[:])

    nc.compile()
    return nc


def prep_inputs(inputs_np, gru_W, gru_U, gru_b, att_W1, att_b1, att_W2, att_b2,
                att_W3, att_b3, au_Wu, au_bu, au_Uu, au_Wr, au_br, au_Ur,
                au_Wc, au_bc, au_Uc, bn_gamma, bn_beta, bn_mean, bn_var,
                d_W1, d_b1, d_W2, d_b2, f_W, f_b):
    """Host-side preprocessing. Returns per-core input maps."""
    f32 = np.float32

    biases = np.zeros((P, 16), f32)
    biases[:, B1H] = gru_b[1, 2 * U : 3 * U]
    biases[:, B0H] = gru_b[0, 2 * U : 3 * U]
    biases[:, BU] = au_bu
    biases[:, BR2] = au_br
    biases[:, BC] = au_bc
    biases[:, B1] = att_b1
    biases[0:64, B2] = att_b2
    biases[0:4, B3] = att_b3[0]

    def dr_pair(a, b):
        out = np.zeros((P, 2, a.shape[1]), NP_F8)
        out[:, 0, :] = a.astype(NP_F8)
        out[:, 1, :] = b.astype(NP_F8)
        return out

    bz = gru_b[0, 0:U] + gru_b[1, 0:U]
    br = gru_b[0, U : 2 * U] + gru_b[1, U : 2 * U]
    rider_z = np.zeros((P, P), f32)
    rider_z[0, :] = bz
    rider_r = np.zeros((P, P), f32)
    rider_r[0, :] = br
    ident = np.eye(P, dtype=f32)

    ax = att_W1[0:P] - att_W1[P : 2 * P]
    am = att_W1[3 * P : 4 * P]
    abc = att_W1[P : 2 * P] + att_W1[2 * P : 3 * P]

    a2pair = np.zeros((P, 2, 64), NP_F8)
    a2pair[:, 0, :] = att_W2.astype(NP_F8)

    # pa3: per k4, (w3 one-hot column k4 | 0) as [64, 2, 4] blocks -> [64, 8, 4]
    a3pair = np.zeros((64, 8, 4), NP_F8)
    for k in range(4):
        col = np.zeros((64, 4), f32)
        col[:, k] = att_W3[:, 0]
        a3pair[:, 2 * k, :] = col.astype(NP_F8)

    # BN folded into head layer 1
    s = (bn_gamma / np.sqrt(bn_var + 1e-3)).astype(f32)
    tt = (bn_beta - bn_mean * s).astype(f32)
    W1p = (s[:, None] * d_W1).astype(f32)
    b1p = (tt @ d_W1 + d_b1).astype(f32)
    biases[:, DB1A] = b1p[0:P]
    biases[:, DB1B] = b1p[P : 2 * P]
    biases[:, DB2] = d_b2
    biases[0, FB] = f_b[0]

    shared = {
        "w_z8": dr_pair(gru_W[:, 0:U], rider_z),
        "w_r8": dr_pair(gru_W[:, U : 2 * U], rider_r),
        "w_c8": dr_pair(ident, gru_W[:, 2 * U : 3 * U]),
        "w_vc8": dr_pair(ident, np.zeros((P, P), f32)),
        "w_a28": a2pair,
        "w_an8": dr_pair(ident, ident),
        "w_a38": a3pair,
        "w_gu": np.ascontiguousarray(gru_U, f32).astype(NP_BF),
        "w_aw": np.ascontiguousarray(
            np.concatenate([au_Wu, au_Wr, au_Wc], axis=1), f32
        ).astype(NP_BF),
        "w_au": np.ascontiguousarray(
            np.concatenate([au_Uu, au_Ur, au_Uc], axis=1), f32
        ).astype(NP_BF),
        "w_ax": np.ascontiguousarray(ax, f32).astype(NP_BF),
        "w_am": np.ascontiguousarray(am, f32).astype(NP_BF),
        "w_d1a": np.ascontiguousarray(W1p[0:P], f32),
        "w_d1b": np.ascontiguousarray(W1p[P : 2 * P], f32),
        "w_d2a": np.ascontiguousarray(d_W2[0:P], f32),
        "w_d2b": np.ascontiguousarray(d_W2[P : 2 * P], f32),
        "w_f": np.ascontiguousarray(f_W, f32),
        "biases": biases,
    }

    in_maps = []
    for cidx in range(NCORES):
        sh = inputs_np[cidx * BL : (cidx + 1) * BL]  # [BL, T+1, D]
        hist_t = np.ascontiguousarray(sh[:, :T, :].transpose(1, 2, 0), f32)
        news_t = np.ascontiguousarray(sh[:, T, :].T, f32)  # [D, BL]
        # anews = abc^T @ news  (per-core, hi+lo fp8 pair)
        anews = (abc.T @ news_t).astype(f32)  # [P, BL]
        an_hi = anews.astype(NP_F8)
        an_lo = (anews - an_hi.astype(f32)).astype(NP_F8)
        an_pair = np.zeros((P, 2, BL), NP_F8)
        an_pair[:, 0, :] = an_hi
        an_pair[:, 1, :] = an_lo
        m = dict(shared)
        m["hist8"] = hist_t.astype(NP_F8)
        m["news_b"] = news_t.astype(NP_BF)
        m["news_r"] = news_t
        m["anews8"] = an_pair
        in_maps.append(m)
    return in_maps


_NC_CACHE = {}


def get_nc(debug=False):
    key = (debug,)
    if key not in _NC_CACHE:
        _NC_CACHE[key] = build_nc(debug=debug)
    return _NC_CACHE[key]


def kernel(**inputs):
    inputs = {k: np.asarray(v) for k, v in inputs.items()}
    in_maps = prep_inputs(
        inputs["inputs"], inputs["gru_W"], inputs["gru_U"], inputs["gru_b"],
        inputs["att_W1"], inputs["att_b1"], inputs["att_W2"], inputs["att_b2"],
        inputs["att_W3"], inputs["att_b3"], inputs["au_Wu"], inputs["au_bu"],
        inputs["au_Uu"], inputs["au_Wr"], inputs["au_br"], inputs["au_Ur"],
        inputs["au_Wc"], inputs["au_bc"], inputs["au_Uc"], inputs["bn_gamma"],
        inputs["bn_beta"], inputs["bn_mean"], inputs["bn_var"], inputs["d_W1"],
        inputs["d_b1"], inputs["d_W2"], inputs["d_b2"], inputs["f_W"],
        inputs["f_b"],
    )
    nc = get_nc(debug=CFG["debug"])
    res = run_bass_kernel_spmd(nc, in_maps, list(range(NCORES)))
    y = np.concatenate(
        [res.results[c]["y"].reshape(-1)[:, None] for c in range(NCORES)], axis=0
    ).astype(np.float32)
    return y


# revision 6
# speedup vs baseline: 1.0327x; 1.0327x over previous
"""DIEN kernel for Trainium2 (Bass/Tile), 8-way data-parallel over batch.

Layout: transposed activations [feature (<=128 partitions), batch (free dim)].
Per core: 512 batch rows, T=50 steps. GRU / attention / AUGRU fused in one
skewed loop (ATT 1 step behind GRU, AUGRU SKEW_AU behind), head at the end.

v2 design (mixed precision, engine-balanced):
- x-side projections run as fp8e4 DoubleRow matmuls (0.5 cyc/row): the host
  quantizes hist to fp8, and each gate's bias rides the DR pair's second slot
  against a static ones vector (stationary row-0 = bias row), so gate psums
  arrive bias-included with zero extra ops.
- candidate folds: t1 = (phh+b1h)*r is an STT (no DVE fast mode either way)
  written directly as fp8 into the moving pair next to x8; one DR (I | Wh)
  then computes xh + t1 in a single 107ns matmul.
- recurrent state h/h2/hs stays bf16 so every combine TT gets the DVE 2x
  mode; h-side matmuls are plain bf16 (1 cyc/row, same as f32r).
- attention: relu1/relu2 emit fp8 directly, pa2/pa3 are DR-padded; the
  abc@news term is precomputed on host (hi+lo fp8 pair, folded via DR(I|I)).
- ats broadcast via gpsimd partition_broadcast (no PSUM bank, no PE).
- PSUM banks (8): zr[2] ur[2] cand[1] hside[1] pa1[1] pa2+a3 shared[1].
"""
import sys

sys.path.insert(0, "/opt/trn_rl_repo")

import numpy as np
import ml_dtypes

import concourse.bass as bass
import concourse.mybir as mybir
import concourse.tile as tile
from concourse import bacc
from concourse.bass_utils import run_bass_kernel_spmd

B, T, D, U = 4096, 50, 128, 128
NCORES = 8
BL = B // NCORES  # 512
P = 128
F32 = mybir.dt.float32
F32R = mybir.dt.float32r
BF16 = mybir.dt.bfloat16
F8 = mybir.dt.float8e4
NP_F8 = ml_dtypes.float8_e4m3
NP_BF = ml_dtypes.bfloat16
AF = mybir.ActivationFunctionType
OP = mybir.AluOpType
DR = mybir.MatmulPerfMode.DoubleRow
LEAKY = 0.0003

CFG = {
    "skew_au": 6,
    # engine per op: "v" = vector(DVE), "g" = gpsimd(Pool)
    "eng": {
        "t1": "v", "t2": "g", "mmj": "v",
        "d": "v", "w": "v", "hn": "v",
        "u_": "v", "d2": "v", "m2": "v", "hn2": "v",
        "relu1": "g", "relu2": "g", "patb": "g",
    },
    "debug": False,
}

# bias column indices in the packed [128, 16] bias tensor
B1H, B0H, BU, BR2, BC, B1, B2, B3, DB1A, DB1B, DB2, FB = range(12)


def _eng(nc, key):
    return nc.vector if CFG["eng"][key] == "v" else nc.gpsimd


def build_nc(debug=False):
    nc = bacc.Bacc()
    SKEW_AU = CFG["skew_au"]
    NITER = T + SKEW_AU

    # ---------------- DRAM inputs
    hist8 = nc.dram_tensor("hist8", [T, P, BL], F8, kind="ExternalInput")
    news_b = nc.dram_tensor("news_b", [P, BL], BF16, kind="ExternalInput")
    news_r = nc.dram_tensor("news_r", [P, BL], F32R, kind="ExternalInput")
    # fp8 DR stationary pairs [128, 2, 128]: (A on slot0, B on slot1)
    w_z8 = nc.dram_tensor("w_z8", [P, 2, P], F8, kind="ExternalInput")  # (Wz | e0*bz)
    w_r8 = nc.dram_tensor("w_r8", [P, 2, P], F8, kind="ExternalInput")  # (Wr | e0*br)
    w_c8 = nc.dram_tensor("w_c8", [P, 2, P], F8, kind="ExternalInput")  # (I  | Wh)
    w_vc8 = nc.dram_tensor("w_vc8", [P, 2, P], F8, kind="ExternalInput")  # (I | 0) AUGRU fold
    w_a28 = nc.dram_tensor("w_a28", [P, 2, 64], F8, kind="ExternalInput")  # (a2w | 0)
    w_an8 = nc.dram_tensor("w_an8", [P, 2, P], F8, kind="ExternalInput")  # (I | I) anews fold
    w_a38 = nc.dram_tensor("w_a38", [64, 8, 4], F8, kind="ExternalInput")  # 4x (w3col | 0)
    # bf16 weights (h-side + attention + AUGRU x-side)
    w_gu = nc.dram_tensor("w_gu", [P, 3 * U], BF16, kind="ExternalInput")
    w_aw = nc.dram_tensor("w_aw", [P, 3 * U], BF16, kind="ExternalInput")
    w_au = nc.dram_tensor("w_au", [P, 3 * U], BF16, kind="ExternalInput")
    w_ax = nc.dram_tensor("w_ax", [P, P], BF16, kind="ExternalInput")
    w_am = nc.dram_tensor("w_am", [P, P], BF16, kind="ExternalInput")
    anews8 = nc.dram_tensor("anews8", [P, 2, BL], F8, kind="ExternalInput")  # (hi | lo)
    # head (f32r as baseline)
    w_d1a = nc.dram_tensor("w_d1a", [P, 256], F32R, kind="ExternalInput")
    w_d1b = nc.dram_tensor("w_d1b", [P, 256], F32R, kind="ExternalInput")
    w_d2a = nc.dram_tensor("w_d2a", [P, P], F32R, kind="ExternalInput")
    w_d2b = nc.dram_tensor("w_d2b", [P, P], F32R, kind="ExternalInput")
    w_f = nc.dram_tensor("w_f", [P, 1], F32R, kind="ExternalInput")
    biases = nc.dram_tensor("biases", [P, 16], F32, kind="ExternalInput")
    y_out = nc.dram_tensor("y", [1, BL], F32, kind="ExternalOutput")
    if debug:
        hg_out = nc.dram_tensor("hg", [P, BL], F32, kind="ExternalOutput")
        h2_out = nc.dram_tensor("h2f", [P, BL], F32, kind="ExternalOutput")

    with tile.TileContext(nc) as tc:
        import contextlib

        ctx = contextlib.ExitStack()
        with ctx:
            wp = ctx.enter_context(tc.tile_pool(name="wp", bufs=1))
            ps = ctx.enter_context(tc.tile_pool(name="ps", bufs=1, space="PSUM"))

            # ---------------- load weights
            def wtile(name, dram, shape, dt):
                t = wp.tile(shape, dt, name=name, tag=name)
                nc.sync.dma_start(t[:], dram[:])
                return t

            z8 = wtile("z8", w_z8, [P, 2, P], F8)
            r8 = wtile("r8", w_r8, [P, 2, P], F8)
            c8 = wtile("c8", w_c8, [P, 2, P], F8)
            vc8 = wtile("vc8", w_vc8, [P, 2, P], F8)
            a28 = wtile("a28", w_a28, [P, 2, 64], F8)
            an8 = wtile("an8", w_an8, [P, 2, P], F8)
            a38 = wtile("a38", w_a38, [64, 8, 4], F8)
            gu = wtile("gu", w_gu, [P, 3 * U], BF16)
            aw = wtile("aw", w_aw, [P, 3 * U], BF16)
            au = wtile("au", w_au, [P, 3 * U], BF16)
            ax = wtile("ax", w_ax, [P, P], BF16)
            am = wtile("am", w_am, [P, P], BF16)
            anp = wtile("anp", anews8, [P, 2, BL], F8)
            d1a = wtile("d1a", w_d1a, [P, 256], F32R)
            d1b = wtile("d1b", w_d1b, [P, 256], F32R)
            d2a = wtile("d2a", w_d2a, [P, P], F32R)
            d2b = wtile("d2b", w_d2b, [P, P], F32R)
            fw = wtile("fw", w_f, [P, 1], F32R)
            bia = wtile("bia", biases, [P, 16], F32)
            newsb = wtile("newsb", news_b, [P, BL], BF16)
            newsr = wtile("newsr", news_r, [P, BL], F32R)

            def bap(col, rows=P):
                return bia[0:rows, col : col + 1]

            # ---------------- static SBUF rings (manual)
            NG = 3
            grings = []  # [t18 | x8 | ones8] fp8
            for k in range(NG):
                g = wp.tile([P, 3, BL], F8, name=f"gring{k}", tag=f"gring{k}")
                nc.vector.memset(g[:, 2, :], 1.0)
                grings.append(g)
            vrings = []  # [t28 | zeros] fp8
            for k in range(2):
                v = wp.tile([P, 2, BL], F8, name=f"vring{k}", tag=f"vring{k}")
                nc.vector.memset(v[:, 1, :], 0.0)
                vrings.append(v)
            a1rings = []  # [a1 | zeros] fp8
            for k in range(2):
                a1r = wp.tile([P, 2, BL], F8, name=f"a1ring{k}", tag=f"a1ring{k}")
                nc.vector.memset(a1r[:, 1, :], 0.0)
                a1rings.append(a1r)
            a2rings = []  # [a2 | zeros] fp8 (64 partitions)
            for k in range(2):
                a2r = wp.tile([64, 2, BL], F8, name=f"a2ring{k}", tag=f"a2ring{k}")
                nc.vector.memset(a2r[:, 1, :], 0.0)
                a2rings.append(a2r)

            NH = 8
            hs_ring = []  # GRU outputs bf16
            for k in range(NH):
                h = wp.tile([P, BL], BF16, name=f"hs{k}", tag=f"hs{k}")
                hs_ring.append(h)
            h2_ring = []
            for k in range(2):
                h2 = wp.tile([P, BL], BF16, name=f"h2_{k}", tag=f"h2_{k}")
                h2_ring.append(h2)
            h_init = wp.tile([P, BL], BF16, name="h_init", tag="h_init")
            nc.vector.memset(h_init[:], 0.0)
            h2_init = wp.tile([P, BL], BF16, name="h2_init", tag="h2_init")
            nc.vector.memset(h2_init[:], 0.0)

            # per-iter small tiles via pools (auto ring by tag)
            sp = ctx.enter_context(tc.tile_pool(name="sp", bufs=2))

            # PSUM banks (8): zr[2] (GRU z|r, then AUGRU u|r2 time-muxed),
            # cand[2] (pxh/pxc alternate), hside[2] (phh/prc alternate),
            # pa1[1], pa2 shares with a3? no: pa2[?]... a3[1]
            def ps_tile(name, shape, tag, bufs=1):
                return ps.tile(shape, F32, name=name, tag=tag, bufs=bufs)

            mm = nc.tensor.matmul

            hs_at = lambda t: h_init if t < 0 else hs_ring[t % NH]
            h2_at = lambda s: h2_init if s < 0 else h2_ring[s % 2]

            ats_tiles = {}
            a3_psums = {}

            # prefetch x8 for t=0
            nc.sync.dma_start(grings[0][:, 1, :], hist8[0])

            for i in range(NITER):
                t = i if i < T else None
                j = i - 1 if 0 <= i - 1 < T else None
                s = i - SKEW_AU if 0 <= i - SKEW_AU < T else None

                # ---------------- GRU step t
                if t is not None:
                    G = grings[t % NG]
                    if t + 1 < T:
                        nc.sync.dma_start(
                            grings[(t + 1) % NG][:, 1, :], hist8[t + 1]
                        )
                    h_prev = hs_at(t - 1)

                    pzr = ps_tile(f"pzr{t}", [P, 2 * BL], "zr")
                    # z: DR(Wz|bz-rider) on (x8, ones8) + bf16 Uz@h
                    mm(pzr[:, 0:BL], z8[:], G[:, 1:3, :], start=True, stop=False,
                       perf_mode=DR)
                    mm(pzr[:, 0:BL], gu[:, 0:U], h_prev[:], start=False, stop=True)
                    mm(pzr[:, BL : 2 * BL], r8[:], G[:, 1:3, :], start=True,
                       stop=False, perf_mode=DR)
                    mm(pzr[:, BL : 2 * BL], gu[:, U : 2 * U], h_prev[:],
                       start=False, stop=True)
                    zr = sp.tile([P, 2 * BL], BF16, name=f"zr{t}", tag="zr_sb")
                    nc.scalar.activation(zr[:], pzr[:], AF.Sigmoid)
                    z, r = zr[:, 0:BL], zr[:, BL : 2 * BL]

                    phh = ps_tile(f"phh{t}", [P, BL], "hside")
                    mm(phh[:], gu[:, 2 * U : 3 * U], h_prev[:], start=True, stop=True)
                    # t18 = (phh + b1h) * r -> fp8 into G slot 0
                    _eng(nc, "t1").scalar_tensor_tensor(
                        G[:, 0, :], phh[:], bap(B1H), r, OP.add, OP.mult
                    )
                    pxh = ps_tile(f"pxh{t}", [P, BL], "cand")
                    mm(pxh[:], c8[:], G[:, 0:2, :], start=True, stop=True,
                       perf_mode=DR)
                    hc = sp.tile([P, BL], BF16, name=f"hc{t}", tag="hc")
                    nc.scalar.activation(hc[:], pxh[:], AF.Tanh, bias=bap(B0H))

                    # hn = hc + z*(h - hc)
                    d = sp.tile([P, BL], BF16, name=f"d{t}", tag="d")
                    _eng(nc, "d").tensor_sub(d[:], h_prev[:], hc[:])
                    w_ = sp.tile([P, BL], BF16, name=f"w{t}", tag="w")
                    _eng(nc, "w").tensor_mul(w_[:], z, d[:])
                    hn = hs_ring[t % NH]
                    _eng(nc, "hn").tensor_add(hn[:], hc[:], w_[:])

                # ---------------- attention step j
                if j is not None:
                    hs_j = hs_at(j)
                    mmj = sp.tile([P, BL], BF16, name=f"mmj{j}", tag="mmj")
                    _eng(nc, "mmj").tensor_mul(mmj[:], hs_j[:], newsb[:])
                    pa1 = ps_tile(f"pa1{j}", [P, BL], "pa1")
                    mm(pa1[:], an8[:], anp[:], start=True, stop=False, perf_mode=DR)
                    mm(pa1[:], ax[:], hs_j[:], start=False, stop=False)
                    mm(pa1[:], am[:], mmj[:], start=False, stop=True)
                    A1 = a1rings[j % 2]
                    _eng(nc, "relu1").tensor_scalar(
                        A1[:, 0, :], pa1[:], bap(B1), 0.0, OP.add, OP.max
                    )
                    pa2 = ps_tile(f"pa2{j}", [64, BL], "pa1")
                    mm(pa2[:], a28[:], A1[:, 0:2, :], start=True, stop=True,
                       perf_mode=DR)
                    A2 = a2rings[j % 2]
                    _eng(nc, "relu2").tensor_scalar(
                        A2[:, 0, :], pa2[:], bap(B2, rows=64), 0.0, OP.add, OP.max
                    )
                    g = j // 4
                    k4 = j % 4
                    if k4 == 0:
                        a3_psums[g] = ps.tile([4, BL], F32, name=f"pa3{g}", tag="a3")
                    mm(a3_psums[g][:], a38[:, 2 * k4 : 2 * k4 + 2, :],
                       A2[:, 0:2, :], start=(k4 == 0),
                       stop=(k4 == 3 or j == T - 1), perf_mode=DR)
                    if k4 == 3 or j == T - 1:
                        k = k4 + 1
                        ats = sp.tile([4, BL], BF16, name=f"ats{g}", tag="ats")
                        nc.scalar.activation(
                            ats[0:k, :], a3_psums[g][0:k, :], AF.Sigmoid,
                            bias=bap(B3, rows=k),
                        )
                        ats_tiles[g] = ats

                # ---------------- AUGRU step s
                if s is not None:
                    V = vrings[s % 2]
                    h2_prev = h2_at(s - 1)
                    hs_s = hs_at(s)

                    pat = sp.tile([P, BL], BF16, name=f"pat{s}", tag="pat")
                    nc.gpsimd.partition_broadcast(
                        pat[:], ats_tiles[s // 4][s % 4 : s % 4 + 1, :]
                    )

                    pur = ps_tile(f"pur{s}", [P, 2 * BL], "ur")
                    mm(pur[:, 0:BL], aw[:, 0:U], hs_s[:], start=True, stop=False)
                    mm(pur[:, 0:BL], au[:, 0:U], h2_prev[:], start=False, stop=True)
                    mm(pur[:, BL : 2 * BL], aw[:, U : 2 * U], hs_s[:],
                       start=True, stop=False)
                    mm(pur[:, BL : 2 * BL], au[:, U : 2 * U], h2_prev[:],
                       start=False, stop=True)
                    ut = sp.tile([P, BL], BF16, name=f"u{s}", tag="u_sb")
                    nc.scalar.activation(ut[:], pur[:, 0:BL], AF.Sigmoid,
                                         bias=bap(BU))
                    r2t = sp.tile([P, BL], BF16, name=f"r2{s}", tag="r2_sb")
                    nc.scalar.activation(r2t[:], pur[:, BL : 2 * BL], AF.Sigmoid,
                                         bias=bap(BR2))

                    prc = ps_tile(f"prc{s}", [P, BL], "hside")
                    mm(prc[:], au[:, 2 * U : 3 * U], h2_prev[:], start=True,
                       stop=True)
                    # t28 = prc * r2 -> fp8 into V slot 0
                    _eng(nc, "t2").scalar_tensor_tensor(
                        V[:, 0, :], prc[:], 0.0, r2t[:], OP.add, OP.mult
                    )
                    pxc = ps_tile(f"pxc{s}", [P, BL], "cand")
                    mm(pxc[:], aw[:, 2 * U : 3 * U], hs_s[:], start=True,
                       stop=False)
                    mm(pxc[:], vc8[:], V[:, 0:2, :], start=False, stop=True,
                       perf_mode=DR)
                    c = sp.tile([P, BL], BF16, name=f"c{s}", tag="c")
                    nc.scalar.activation(c[:], pxc[:], AF.Tanh, bias=bap(BC))

                    # hn2 = h2 + u_*(c - h2), u_ = pat*u
                    u_ = sp.tile([P, BL], BF16, name=f"u_{s}", tag="u_")
                    _eng(nc, "u_").tensor_mul(u_[:], pat[:], ut[:])
                    d2 = sp.tile([P, BL], BF16, name=f"d2{s}", tag="d2")
                    _eng(nc, "d2").tensor_sub(d2[:], c[:], h2_prev[:])
                    m2 = sp.tile([P, BL], BF16, name=f"m2{s}", tag="m2")
                    _eng(nc, "m2").tensor_mul(m2[:], u_[:], d2[:])
                    hn2 = h2_ring[s % 2]
                    _eng(nc, "hn2").tensor_add(hn2[:], h2_prev[:], m2[:])

            # ---------------- deep head (one-time, f32r)
            h2f = h2_ring[(T - 1) % 2]
            if debug:
                hgf = sp.tile([P, BL], F32, name="hgf", tag="hgf")
                nc.scalar.copy(hgf[:], hs_ring[(T - 1) % NH][:])
                nc.sync.dma_start(hg_out[:], hgf[:])
                h2c = sp.tile([P, BL], F32, name="h2c", tag="h2c")
                nc.scalar.copy(h2c[:], h2f[:])
                nc.sync.dma_start(h2_out[:], h2c[:])

            o1 = sp.tile([P, 2 * BL], F32R, name="o1", tag="o1")
            for mch in range(2):
                po = ps_tile(f"po1_{mch}", [P, BL], "cand")
                mm(po[:], d1a[:, mch * P : (mch + 1) * P], h2f[:], start=True,
                   stop=False)
                mm(po[:], d1b[:, mch * P : (mch + 1) * P], newsr[:], start=False,
                   stop=True)
                nc.scalar.activation(
                    o1[:, mch * BL : (mch + 1) * BL], po[:], AF.Lrelu,
                    bias=bap(DB1A + mch), alpha=LEAKY,
                )
            po2 = ps_tile("po2", [P, BL], "hside")
            mm(po2[:], d2a[:], o1[:, 0:BL], start=True, stop=False)
            mm(po2[:], d2b[:], o1[:, BL : 2 * BL], start=False, stop=True)
            o2 = sp.tile([P, BL], F32R, name="o2", tag="o2")
            nc.scalar.activation(o2[:], po2[:], AF.Lrelu, bias=bap(DB2),
                                 alpha=LEAKY)
            py = ps_tile("py", [1, BL], "pa1")
            mm(py[:], fw[:], o2[:], start=True, stop=True)
            y_sb = sp.tile([1, BL], F32, name="y_sb", tag="ysb")
            nc.scalar.activation(y_sb[:], py[:], AF.Sigmoid, bias=bap(FB, rows=1))
            nc.sync.dma_start(y_out[:], y_sb[:])

    nc.compile()
    return nc


def prep_inputs(inputs_np, gru_W, gru_U, gru_b, att_W1, att_b1, att_W2, att_b2,
                att_W3, att_b3, au_Wu, au_bu, au_Uu, au_Wr, au_br, au_Ur,
                au_Wc, au_bc, au_Uc, bn_gamma, bn_beta, bn_mean, bn_var,
                d_W1, d_b1, d_W2, d_b2, f_W, f_b):
    """Host-side preprocessing. Returns per-core input maps."""
    f32 = np.float32

    biases = np.zeros((P, 16), f32)
    biases[:, B1H] = gru_b[1, 2 * U : 3 * U]
    biases[:, B0H] = gru_b[0, 2 * U : 3 * U]
    biases[:, BU] = au_bu
    biases[:, BR2] = au_br
    biases[:, BC] = au_bc
    biases[:, B1] = att_b1
    biases[0:64, B2] = att_b2
    biases[0:4, B3] = att_b3[0]

    def dr_pair(a, b):
        out = np.zeros((P, 2, a.shape[1]), NP_F8)
        out[:, 0, :] = a.astype(NP_F8)
        out[:, 1, :] = b.astype(NP_F8)
        return out

    bz = gru_b[0, 0:U] + gru_b[1, 0:U]
    br = gru_b[0, U : 2 * U] + gru_b[1, U : 2 * U]
    rider_z = np.zeros((P, P), f32)
    rider_z[0, :] = bz
    rider_r = np.zeros((P, P), f32)
    rider_r[0, :] = br
    ident = np.eye(P, dtype=f32)

    ax = att_W1[0:P] - att_W1[P : 2 * P]
    am = att_W1[3 * P : 4 * P]
    abc = att_W1[P : 2 * P] + att_W1[2 * P : 3 * P]

    a2pair = np.zeros((P, 2, 64), NP_F8)
    a2pair[:, 0, :] = att_W2.astype(NP_F8)

    # pa3: per k4, (w3 one-hot column k4 | 0) as [64, 2, 4] blocks -> [64, 8, 4]
    a3pair = np.zeros((64, 8, 4), NP_F8)
    for k in range(4):
        col = np.zeros((64, 4), f32)
        col[:, k] = att_W3[:, 0]
        a3pair[:, 2 * k, :] = col.astype(NP_F8)

    # BN folded into head layer 1
    s = (bn_gamma / np.sqrt(bn_var + 1e-3)).astype(f32)
    tt = (bn_beta - bn_mean * s).astype(f32)
    W1p = (s[:, None] * d_W1).astype(f32)
    b1p = (tt @ d_W1 + d_b1).astype(f32)
    biases[:, DB1A] = b1p[0:P]
    biases[:, DB1B] = b1p[P : 2 * P]
    biases[:, DB2] = d_b2
    biases[0, FB] = f_b[0]

    shared = {
        "w_z8": dr_pair(gru_W[:, 0:U], rider_z),
        "w_r8": dr_pair(gru_W[:, U : 2 * U], rider_r),
        "w_c8": dr_pair(ident, gru_W[:, 2 * U : 3 * U]),
        "w_vc8": dr_pair(ident, np.zeros((P, P), f32)),
        "w_a28": a2pair,
        "w_an8": dr_pair(ident, ident),
        "w_a38": a3pair,
        "w_gu": np.ascontiguousarray(gru_U, f32).astype(NP_BF),
        "w_aw": np.ascontiguousarray(
            np.concatenate([au_Wu, au_Wr, au_Wc], axis=1), f32
        ).astype(NP_BF),
        "w_au": np.ascontiguousarray(
            np.concatenate([au_Uu, au_Ur, au_Uc], axis=1), f32
        ).astype(NP_BF),
        "w_ax": np.ascontiguousarray(ax, f32).astype(NP_BF),
        "w_am": np.ascontiguousarray(am, f32).astype(NP_BF),
        "w_d1a": np.ascontiguousarray(W1p[0:P], f32),
        "w_d1b": np.ascontiguousarray(W1p[P : 2 * P], f32),
        "w_d2a": np.ascontiguousarray(d_W2[0:P], f32),
        "w_d2b": np.ascontiguousarray(d_W2[P : 2 * P], f32),
        "w_f": np.ascontiguousarray(f_W, f32),
        "biases": biases,
    }

    in_maps = []
    for cidx in range(NCORES):
        sh = inputs_np[cidx * BL : (cidx + 1) * BL]  # [BL, T+1, D]
        hist_t = np.ascontiguousarray(sh[:, :T, :].transpose(1, 2, 0), f32)
        news_t = np.ascontiguousarray(sh[:, T, :].T, f32)  # [D, BL]
        # anews = abc^T @ news  (per-core, hi+lo fp8 pair)
        anews = (abc.T @ news_t).astype(f32)  # [P, BL]
        an_hi = anews.astype(NP_F8)
        an_lo = (anews - an_hi.astype(f32)).astype(NP_F8)
        an_pair = np.zeros((P, 2, BL), NP_F8)
        an_pair[:, 0, :] = an_hi
        an_pair[:, 1, :] = an_lo
        m = dict(shared)
        m["hist8"] = hist_t.astype(NP_F8)
        m["news_b"] = news_t.astype(NP_BF)
        m["news_r"] = news_t
        m["anews8"] = an_pair
        in_maps.append(m)
    return in_maps


_NC_CACHE = {}


def get_nc(debug=False):
    key = (debug,)
    if key not in _NC_CACHE:
        _NC_CACHE[key] = build_nc(debug=debug)
    return _NC_CACHE[key]


def kernel(**inputs):
    inputs = {k: np.asarray(v) for k, v in inputs.items()}
    in_maps = prep_inputs(
        inputs["inputs"], inputs["gru_W"], inputs["gru_U"], inputs["gru_b"],
        inputs["att_W1"], inputs["att_b1"], inputs["att_W2"], inputs["att_b2"],
        inputs["att_W3"], inputs["att_b3"], inputs["au_Wu"], inputs["au_bu"],
        inputs["au_Uu"], inputs["au_Wr"], inputs["au_br"], inputs["au_Ur"],
        inputs["au_Wc"], inputs["au_bc"], inputs["au_Uc"], inputs["bn_gamma"],
        inputs["bn_beta"], inputs["bn_mean"], inputs["bn_var"], inputs["d_W1"],
        inputs["d_b1"], inputs["d_W2"], inputs["d_b2"], inputs["f_W"],
        inputs["f_b"],
    )
    nc = get_nc(debug=CFG["debug"])
    res = run_bass_kernel_spmd(nc, in_maps, list(range(NCORES)))
    y = np.concatenate(
        [res.results[c]["y"].reshape(-1)[:, None] for c in range(NCORES)], axis=0
    ).astype(np.float32)
    return y


# revision 18
# speedup vs baseline: 1.0423x; 1.0094x over previous
"""DIEN kernel for Trainium2 (Bass/Tile), 8-way data-parallel over batch.

Layout: transposed activations [feature (<=128 partitions), batch (free dim)].
Per core: 512 batch rows, T=50 steps. GRU / attention / AUGRU fused in one
skewed loop (ATT 1 step behind GRU, AUGRU SKEW_AU behind), head at the end.

v2 design (mixed precision, engine-balanced):
- x-side projections run as fp8e4 DoubleRow matmuls (0.5 cyc/row): the host
  quantizes hist to fp8, and each gate's bias rides the DR pair's second slot
  against a static ones vector (stationary row-0 = bias row), so gate psums
  arrive bias-included with zero extra ops.
- candidate folds: t1 = (phh+b1h)*r is an STT (no DVE fast mode either way)
  written directly as fp8 into the moving pair next to x8; one DR (I | Wh)
  then computes xh + t1 in a single 107ns matmul.
- recurrent state h/h2/hs stays bf16 so every combine TT gets the DVE 2x
  mode; h-side matmuls are plain bf16 (1 cyc/row, same as f32r).
- attention: relu1/relu2 emit fp8 directly, pa2/pa3 are DR-padded; the
  abc@news term is precomputed on host (hi+lo fp8 pair, folded via DR(I|I)).
- ats broadcast via gpsimd partition_broadcast (no PSUM bank, no PE).
- PSUM banks (8): zr[2] ur[2] cand[1] hside[1] pa1[1] pa2+a3 shared[1].
"""
import sys

sys.path.insert(0, "/opt/trn_rl_repo")

import numpy as np
import ml_dtypes

import concourse.bass as bass
import concourse.mybir as mybir
import concourse.tile as tile
from concourse import bacc
from concourse.bass_utils import run_bass_kernel_spmd

B, T, D, U = 4096, 50, 128, 128
NCORES = 8
BL = B // NCORES  # 512
P = 128
F32 = mybir.dt.float32
F32R = mybir.dt.float32r
BF16 = mybir.dt.bfloat16
F8 = mybir.dt.float8e4
NP_F8 = ml_dtypes.float8_e4m3
NP_BF = ml_dtypes.bfloat16
AF = mybir.ActivationFunctionType
OP = mybir.AluOpType
DR = mybir.MatmulPerfMode.DoubleRow
LEAKY = 0.0003

CFG = {
    "skew_au": 6,
    # engine per op: "v" = vector(DVE), "g" = gpsimd(Pool)
    "eng": {
        "t1": "v", "t2": "g", "mmj": "v",
        "d": "v", "w": "v", "hn": "v",
        "u_": "v", "d2": "v", "m2": "v", "hn2": "v",
        "relu1": "g", "relu2": "g", "patb": "g",
    },
    "debug": False,
}

# bias column indices in the packed [128, 16] bias tensor
B1H, B0H, BU, BR2, BC, B1, B2, B3, DB1A, DB1B, DB2, FB = range(12)


def _eng(nc, key):
    return nc.vector if CFG["eng"][key] == "v" else nc.gpsimd


def build_nc(debug=False):
    nc = bacc.Bacc()
    SKEW_AU = CFG["skew_au"]
    NITER = T + SKEW_AU

    # ---------------- DRAM inputs
    hist8 = nc.dram_tensor("hist8", [T, P, BL], F8, kind="ExternalInput")
    news_b = nc.dram_tensor("news_b", [P, BL], BF16, kind="ExternalInput")
    news_r = nc.dram_tensor("news_r", [P, BL], F32R, kind="ExternalInput")
    # fp8 DR stationary pairs [128, 2, 128]: (A on slot0, B on slot1)
    w_z8 = nc.dram_tensor("w_z8", [P, 2, P], F8, kind="ExternalInput")  # (Wz | e0*bz)
    w_r8 = nc.dram_tensor("w_r8", [P, 2, P], F8, kind="ExternalInput")  # (Wr | e0*br)
    w_c8 = nc.dram_tensor("w_c8", [P, 2, P], F8, kind="ExternalInput")  # (I  | Wh)
    w_vc8 = nc.dram_tensor("w_vc8", [P, 2, P], F8, kind="ExternalInput")  # (I | 0) AUGRU fold
    w_a28 = nc.dram_tensor("w_a28", [P, 2, 64], F8, kind="ExternalInput")  # (a2w | 0)
    w_an8 = nc.dram_tensor("w_an8", [P, 2, P], F8, kind="ExternalInput")  # (I | I) anews fold
    w_a38 = nc.dram_tensor("w_a38", [64, 8, 4], F8, kind="ExternalInput")  # 4x (w3col | 0)
    w_bu8 = nc.dram_tensor("w_bu8", [1, 2, P], F8, kind="ExternalInput")  # (0 | bu row)
    # bf16 weights (h-side + attention + AUGRU x-side)
    w_gu = nc.dram_tensor("w_gu", [P, 3 * U], BF16, kind="ExternalInput")
    w_aw = nc.dram_tensor("w_aw", [P, 3 * U], BF16, kind="ExternalInput")
    w_au = nc.dram_tensor("w_au", [P, 3 * U], BF16, kind="ExternalInput")
    w_ax = nc.dram_tensor("w_ax", [P, P], BF16, kind="ExternalInput")
    w_am = nc.dram_tensor("w_am", [P, P], BF16, kind="ExternalInput")
    anews8 = nc.dram_tensor("anews8", [P, 2, BL], F8, kind="ExternalInput")  # (hi | lo)
    # head (f32r as baseline)
    w_d1a = nc.dram_tensor("w_d1a", [P, 256], F32R, kind="ExternalInput")
    w_d1b = nc.dram_tensor("w_d1b", [P, 256], F32R, kind="ExternalInput")
    w_d2a = nc.dram_tensor("w_d2a", [P, P], F32R, kind="ExternalInput")
    w_d2b = nc.dram_tensor("w_d2b", [P, P], F32R, kind="ExternalInput")
    w_f = nc.dram_tensor("w_f", [P, 1], F32R, kind="ExternalInput")
    biases = nc.dram_tensor("biases", [P, 16], F32, kind="ExternalInput")
    y_out = nc.dram_tensor("y", [1, BL], F32, kind="ExternalOutput")
    if debug:
        hg_out = nc.dram_tensor("hg", [P, BL], F32, kind="ExternalOutput")
        h2_out = nc.dram_tensor("h2f", [P, BL], F32, kind="ExternalOutput")

    with tile.TileContext(nc) as tc:
        import contextlib

        ctx = contextlib.ExitStack()
        with ctx:
            wp = ctx.enter_context(tc.tile_pool(name="wp", bufs=1))
            ps = ctx.enter_context(tc.tile_pool(name="ps", bufs=1, space="PSUM"))

            # ---------------- load weights
            def wtile(name, dram, shape, dt):
                t = wp.tile(shape, dt, name=name, tag=name)
                nc.sync.dma_start(t[:], dram[:])
                return t

            z8 = wtile("z8", w_z8, [P, 2, P], F8)
            r8 = wtile("r8", w_r8, [P, 2, P], F8)
            c8 = wtile("c8", w_c8, [P, 2, P], F8)
            vc8 = wtile("vc8", w_vc8, [P, 2, P], F8)
            a28 = wtile("a28", w_a28, [P, 2, 64], F8)
            an8 = wtile("an8", w_an8, [P, 2, P], F8)
            a38 = wtile("a38", w_a38, [64, 8, 4], F8)
            bu8 = wtile("bu8", w_bu8, [1, 2, P], F8)
            gu = wtile("gu", w_gu, [P, 3 * U], BF16)
            aw = wtile("aw", w_aw, [P, 3 * U], BF16)
            au = wtile("au", w_au, [P, 3 * U], BF16)
            ax = wtile("ax", w_ax, [P, P], BF16)
            am = wtile("am", w_am, [P, P], BF16)
            anp = wtile("anp", anews8, [P, 2, BL], F8)
            d1a = wtile("d1a", w_d1a, [P, 256], F32R)
            d1b = wtile("d1b", w_d1b, [P, 256], F32R)
            d2a = wtile("d2a", w_d2a, [P, P], F32R)
            d2b = wtile("d2b", w_d2b, [P, P], F32R)
            fw = wtile("fw", w_f, [P, 1], F32R)
            bia = wtile("bia", biases, [P, 16], F32)
            newsb = wtile("newsb", news_b, [P, BL], BF16)
            newsr = wtile("newsr", news_r, [P, BL], F32R)

            def bap(col, rows=P):
                return bia[0:rows, col : col + 1]

            # ---------------- static SBUF rings (manual)
            NG = 3
            grings = []  # [t18 | x8 | ones8] fp8
            for k in range(NG):
                g = wp.tile([P, 3, BL], F8, name=f"gring{k}", tag=f"gring{k}")
                nc.vector.memset(g[:, 2, :], 1.0)
                grings.append(g)
            vrings = []  # [t28 | zeros] fp8
            for k in range(2):
                v = wp.tile([P, 2, BL], F8, name=f"vring{k}", tag=f"vring{k}")
                nc.vector.memset(v[:, 1, :], 0.0)
                vrings.append(v)
            a1rings = []  # [a1 | zeros] fp8
            for k in range(2):
                a1r = wp.tile([P, 2, BL], F8, name=f"a1ring{k}", tag=f"a1ring{k}")
                nc.vector.memset(a1r[:, 1, :], 0.0)
                a1rings.append(a1r)
            a2rings = []  # [a2 | zeros] fp8 (64 partitions)
            for k in range(2):
                a2r = wp.tile([64, 2, BL], F8, name=f"a2ring{k}", tag=f"a2ring{k}")
                nc.vector.memset(a2r[:, 1, :], 0.0)
                a2rings.append(a2r)
            onesp = wp.tile([1, 2, BL], F8, name="onesp", tag="onesp")
            nc.vector.memset(onesp[:, 0, :], 0.0)
            nc.vector.memset(onesp[:, 1, :], 1.0)

            NH = 8
            hs_ring = []  # GRU outputs bf16
            for k in range(NH):
                h = wp.tile([P, BL], BF16, name=f"hs{k}", tag=f"hs{k}")
                hs_ring.append(h)
            h2_ring = []
            for k in range(2):
                h2 = wp.tile([P, BL], BF16, name=f"h2_{k}", tag=f"h2_{k}")
                h2_ring.append(h2)
            h_init = wp.tile([P, BL], BF16, name="h_init", tag="h_init")
            nc.vector.memset(h_init[:], 0.0)
            h2_init = wp.tile([P, BL], BF16, name="h2_init", tag="h2_init")
            nc.vector.memset(h2_init[:], 0.0)

            # per-iter small tiles via pools (auto ring by tag)
            sp = ctx.enter_context(tc.tile_pool(name="sp", bufs=2))

            # PSUM banks (8): rr2[2] (r|r2 segments), zu[2] (z|u merged
            # sigmoid), gcand[1] (phh then pxh: pxh's DR already waits the
            # t18 read of phh), acand[1] (prc then pxc, vc8-DR is the start),
            # pa1[1], pa2+a3 partition-split in one persistent bank[1].
            def ps_tile(name, shape, tag, bufs=1):
                return ps.tile(shape, F32, name=name, tag=tag, bufs=bufs)

            mm = nc.tensor.matmul

            hs_at = lambda t: h_init if t < 0 else hs_ring[t % NH]
            h2_at = lambda s: h2_init if s < 0 else h2_ring[s % 2]

            ats_tiles = {}

            pa2a3 = ps.tile([P, BL], F32, name="pa2a3", tag="pa2a3")

            # prefetch x8 for t=0
            nc.sync.dma_start(grings[0][:, 1, :], hist8[0])

            for i in range(NITER):
                t = i if i < T else None
                j = i - 1 if 0 <= i - 1 < T else None
                s = i - SKEW_AU if 0 <= i - SKEW_AU < T else None

                G = grings[t % NG] if t is not None else None
                h_prev = hs_at(t - 1) if t is not None else None
                hs_j = hs_at(j) if j is not None else None
                V = vrings[s % 2] if s is not None else None
                h2_prev = h2_at(s - 1) if s is not None else None
                hs_s = hs_at(s) if s is not None else None

                # ===== phase A: iteration-start-ready matmuls (PE) =====
                prr = ps_tile(f"prr{i}", [P, 2 * BL], "rr2")
                pzu = ps_tile(f"pzu{i}", [P, 2 * BL], "zu")
                if t is not None:
                    if t + 1 < T:
                        nc.sync.dma_start(
                            grings[(t + 1) % NG][:, 1, :], hist8[t + 1]
                        )
                    mm(prr[:, 0:BL], r8[:], G[:, 1:3, :], start=True,
                       stop=False, perf_mode=DR)
                    mm(prr[:, 0:BL], gu[:, U : 2 * U], h_prev[:],
                       start=False, stop=True)
                if s is not None:
                    mm(prr[:, BL : 2 * BL], aw[:, U : 2 * U], hs_s[:],
                       start=True, stop=False)
                    mm(prr[:, BL : 2 * BL], au[:, U : 2 * U], h2_prev[:],
                       start=False, stop=True)
                if t is not None:
                    mm(pzu[:, 0:BL], z8[:], G[:, 1:3, :], start=True,
                       stop=False, perf_mode=DR)
                    mm(pzu[:, 0:BL], gu[:, 0:U], h_prev[:], start=False,
                       stop=True)
                if s is not None:
                    mm(pzu[:, BL : 2 * BL], aw[:, 0:U], hs_s[:], start=True,
                       stop=False)
                    mm(pzu[:, BL : 2 * BL], au[:, 0:U], h2_prev[:],
                       start=False, stop=False)
                    # bu bias rider: K=1 DR on (zero-row, ones-row)
                    mm(pzu[:, BL : 2 * BL], bu8[:], onesp[:],
                       start=False, stop=True, perf_mode=DR)
                if t is not None:
                    phh = ps_tile(f"phh{t}", [P, BL], "gcand")
                    mm(phh[:], gu[:, 2 * U : 3 * U], h_prev[:], start=True,
                       stop=True)
                if s is not None:
                    prc = ps_tile(f"prc{s}", [P, BL], "acand")
                    mm(prc[:], au[:, 2 * U : 3 * U], h2_prev[:], start=True,
                       stop=True)

                # ===== ready elementwise: mmj (DVE), patb (Pool) =====
                if j is not None:
                    mmj = sp.tile([P, BL], BF16, name=f"mmj{j}", tag="mmj")
                    _eng(nc, "mmj").tensor_mul(mmj[:], hs_j[:], newsb[:])
                if s is not None:
                    pat = sp.tile([P, BL], BF16, name=f"pat{s}", tag="pat")
                    nc.gpsimd.partition_broadcast(
                        pat[:], ats_tiles[s // 4][s % 4 : s % 4 + 1, :]
                    )

                # ===== chain heads: sigma-r, sigma-r2 (ACT) =====
                if t is not None:
                    rt = sp.tile([P, BL], BF16, name=f"r{t}", tag="r_sb")
                    nc.scalar.activation(rt[:], prr[:, 0:BL], AF.Sigmoid)
                if s is not None:
                    r2t = sp.tile([P, BL], BF16, name=f"r2{s}", tag="r2_sb")
                    nc.scalar.activation(r2t[:], prr[:, BL : 2 * BL],
                                         AF.Sigmoid, bias=bap(BR2))

                # ===== folds: t18 (DVE), t28 (Pool) =====
                if t is not None:
                    _eng(nc, "t1").scalar_tensor_tensor(
                        G[:, 0, :], phh[:], bap(B1H), rt[:], OP.add, OP.mult
                    )
                if s is not None:
                    _eng(nc, "t2").scalar_tensor_tensor(
                        V[:, 0, :], prc[:], 0.0, r2t[:], OP.add, OP.mult
                    )

                # ===== merged sigma-[z|u] (ACT) =====
                zu = sp.tile([P, 2 * BL], BF16, name=f"zu{i}", tag="zu_sb")
                if t is not None and s is not None:
                    nc.scalar.activation(zu[:], pzu[:], AF.Sigmoid)
                elif t is not None:
                    nc.scalar.activation(zu[:, 0:BL], pzu[:, 0:BL], AF.Sigmoid)
                elif s is not None:
                    nc.scalar.activation(zu[:, BL : 2 * BL],
                                         pzu[:, BL : 2 * BL], AF.Sigmoid)
                z = zu[:, 0:BL]
                ut = zu[:, BL : 2 * BL]

                # ===== attention pa1 (PE) + relu1 (Pool) =====
                if j is not None:
                    pa1 = ps_tile(f"pa1{j}", [P, BL], "pa1")
                    mm(pa1[:], an8[:], anp[:], start=True, stop=False,
                       perf_mode=DR)
                    mm(pa1[:], ax[:], hs_j[:], start=False, stop=False)
                    mm(pa1[:], am[:], mmj[:], start=False, stop=True)
                    A1 = a1rings[j % 2]
                    _eng(nc, "relu1").tensor_scalar(
                        A1[:, 0, :], pa1[:], bap(B1), 0.0, OP.add, OP.max
                    )

                # ===== candidates: pxh-DR + tanh-hc, pxc + tanh-c =====
                if t is not None:
                    pxh = ps_tile(f"pxh{t}", [P, BL], "gcand")
                    mm(pxh[:], c8[:], G[:, 0:2, :], start=True, stop=True,
                       perf_mode=DR)
                    hc = sp.tile([P, BL], BF16, name=f"hc{t}", tag="hc")
                    nc.scalar.activation(hc[:], pxh[:], AF.Tanh, bias=bap(B0H))
                if s is not None:
                    pxc = ps_tile(f"pxc{s}", [P, BL], "acand")
                    mm(pxc[:], vc8[:], V[:, 0:2, :], start=True, stop=False,
                       perf_mode=DR)
                    mm(pxc[:], aw[:, 2 * U : 3 * U], hs_s[:], start=False,
                       stop=True)
                    c = sp.tile([P, BL], BF16, name=f"c{s}", tag="c")
                    nc.scalar.activation(c[:], pxc[:], AF.Tanh, bias=bap(BC))

                # ===== combines (DVE): GRU first, then AUGRU =====
                if t is not None:
                    # hn = hc + z*(h - hc)
                    d = sp.tile([P, BL], BF16, name=f"d{t}", tag="d")
                    _eng(nc, "d").tensor_sub(d[:], h_prev[:], hc[:])
                    w_ = sp.tile([P, BL], BF16, name=f"w{t}", tag="w")
                    _eng(nc, "w").tensor_mul(w_[:], z, d[:])
                    hn = hs_ring[t % NH]
                    _eng(nc, "hn").tensor_add(hn[:], hc[:], w_[:])
                if s is not None:
                    # hn2 = h2 + u_*(c - h2), u_ = pat*u
                    u_ = sp.tile([P, BL], BF16, name=f"u_{s}", tag="u_")
                    _eng(nc, "u_").tensor_mul(u_[:], pat[:], ut)
                    d2 = sp.tile([P, BL], BF16, name=f"d2{s}", tag="d2")
                    _eng(nc, "d2").tensor_sub(d2[:], c[:], h2_prev[:])
                    m2 = sp.tile([P, BL], BF16, name=f"m2{s}", tag="m2")
                    _eng(nc, "m2").tensor_mul(m2[:], u_[:], d2[:])
                    hn2 = h2_ring[s % 2]
                    _eng(nc, "hn2").tensor_add(hn2[:], h2_prev[:], m2[:])

                # ===== attention tail: pa2, relu2, pa3, [ats sigma] =====
                if j is not None:
                    mm(pa2a3[0:64, :], a28[:], A1[:, 0:2, :], start=True,
                       stop=True, perf_mode=DR)
                    A2 = a2rings[j % 2]
                    _eng(nc, "relu2").tensor_scalar(
                        A2[:, 0, :], pa2a3[0:64, :], bap(B2, rows=64), 0.0,
                        OP.add, OP.max
                    )
                    g = j // 4
                    k4 = j % 4
                    mm(pa2a3[64:68, :], a38[:, 2 * k4 : 2 * k4 + 2, :],
                       A2[:, 0:2, :], start=(k4 == 0),
                       stop=(k4 == 3 or j == T - 1), perf_mode=DR)
                    if k4 == 3 or j == T - 1:
                        k = k4 + 1
                        ats = sp.tile([4, BL], BF16, name=f"ats{g}", tag="ats")
                        nc.scalar.activation(
                            ats[0:k, :], pa2a3[64 : 64 + k, :], AF.Sigmoid,
                            bias=bap(B3, rows=k),
                        )
                        ats_tiles[g] = ats



            # ---------------- deep head (one-time, f32r)
            h2f = h2_ring[(T - 1) % 2]
            if debug:
                hgf = sp.tile([P, BL], F32, name="hgf", tag="hgf")
                nc.scalar.copy(hgf[:], hs_ring[(T - 1) % NH][:])
                nc.sync.dma_start(hg_out[:], hgf[:])
                h2c = sp.tile([P, BL], F32, name="h2c", tag="h2c")
                nc.scalar.copy(h2c[:], h2f[:])
                nc.sync.dma_start(h2_out[:], h2c[:])

            o1 = sp.tile([P, 2 * BL], F32R, name="o1", tag="o1")
            for mch in range(2):
                po = ps_tile(f"po1_{mch}", [P, BL], "gcand")
                mm(po[:], d1a[:, mch * P : (mch + 1) * P], h2f[:], start=True,
                   stop=False)
                mm(po[:], d1b[:, mch * P : (mch + 1) * P], newsr[:], start=False,
                   stop=True)
                nc.scalar.activation(
                    o1[:, mch * BL : (mch + 1) * BL], po[:], AF.Lrelu,
                    bias=bap(DB1A + mch), alpha=LEAKY,
                )
            po2 = ps_tile("po2", [P, BL], "acand")
            mm(po2[:], d2a[:], o1[:, 0:BL], start=True, stop=False)
            mm(po2[:], d2b[:], o1[:, BL : 2 * BL], start=False, stop=True)
            o2 = sp.tile([P, BL], F32R, name="o2", tag="o2")
            nc.scalar.activation(o2[:], po2[:], AF.Lrelu, bias=bap(DB2),
                                 alpha=LEAKY)
            py = ps_tile("py", [1, BL], "pa1")
            mm(py[:], fw[:], o2[:], start=True, stop=True)
            y_sb = sp.tile([1, BL], F32, name="y_sb", tag="ysb")
            nc.scalar.activation(y_sb[:], py[:], AF.Sigmoid, bias=bap(FB, rows=1))
            nc.sync.dma_start(y_out[:], y_sb[:])

    nc.compile()
    return nc


def prep_inputs(inputs_np, gru_W, gru_U, gru_b, att_W1, att_b1, att_W2, att_b2,
                att_W3, att_b3, au_Wu, au_bu, au_Uu, au_Wr, au_br, au_Ur,
                au_Wc, au_bc, au_Uc, bn_gamma, bn_beta, bn_mean, bn_var,
                d_W1, d_b1, d_W2, d_b2, f_W, f_b):
    """Host-side preprocessing. Returns per-core input maps."""
    f32 = np.float32

    biases = np.zeros((P, 16), f32)
    biases[:, B1H] = gru_b[1, 2 * U : 3 * U]
    biases[:, B0H] = gru_b[0, 2 * U : 3 * U]
    biases[:, BU] = au_bu
    biases[:, BR2] = au_br
    biases[:, BC] = au_bc
    biases[:, B1] = att_b1
    biases[0:64, B2] = att_b2
    biases[0:4, B3] = att_b3[0]

    def dr_pair(a, b):
        out = np.zeros((P, 2, a.shape[1]), NP_F8)
        out[:, 0, :] = a.astype(NP_F8)
        out[:, 1, :] = b.astype(NP_F8)
        return out

    bz = gru_b[0, 0:U] + gru_b[1, 0:U]
    br = gru_b[0, U : 2 * U] + gru_b[1, U : 2 * U]
    rider_z = np.zeros((P, P), f32)
    rider_z[0, :] = bz
    rider_r = np.zeros((P, P), f32)
    rider_r[0, :] = br
    ident = np.eye(P, dtype=f32)

    ax = att_W1[0:P] - att_W1[P : 2 * P]
    am = att_W1[3 * P : 4 * P]
    abc = att_W1[P : 2 * P] + att_W1[2 * P : 3 * P]

    a2pair = np.zeros((P, 2, 64), NP_F8)
    a2pair[:, 0, :] = att_W2.astype(NP_F8)

    # pa3: per k4, (w3 one-hot column k4 | 0) as [64, 2, 4] blocks -> [64, 8, 4]
    a3pair = np.zeros((64, 8, 4), NP_F8)
    for k in range(4):
        col = np.zeros((64, 4), f32)
        col[:, k] = att_W3[:, 0]
        a3pair[:, 2 * k, :] = col.astype(NP_F8)

    bu8 = np.zeros((1, 2, P), NP_F8)
    bu8[0, 1, :] = au_bu.astype(NP_F8)

    # BN folded into head layer 1
    s = (bn_gamma / np.sqrt(bn_var + 1e-3)).astype(f32)
    tt = (bn_beta - bn_mean * s).astype(f32)
    W1p = (s[:, None] * d_W1).astype(f32)
    b1p = (tt @ d_W1 + d_b1).astype(f32)
    biases[:, DB1A] = b1p[0:P]
    biases[:, DB1B] = b1p[P : 2 * P]
    biases[:, DB2] = d_b2
    biases[0, FB] = f_b[0]

    shared = {
        "w_z8": dr_pair(gru_W[:, 0:U], rider_z),
        "w_r8": dr_pair(gru_W[:, U : 2 * U], rider_r),
        "w_c8": dr_pair(ident, gru_W[:, 2 * U : 3 * U]),
        "w_vc8": dr_pair(ident, np.zeros((P, P), f32)),
        "w_a28": a2pair,
        "w_an8": dr_pair(ident, ident),
        "w_a38": a3pair,
        "w_bu8": bu8,
        "w_gu": np.ascontiguousarray(gru_U, f32).astype(NP_BF),
        "w_aw": np.ascontiguousarray(
            np.concatenate([au_Wu, au_Wr, au_Wc], axis=1), f32
        ).astype(NP_BF),
        "w_au": np.ascontiguousarray(
            np.concatenate([au_Uu, au_Ur, au_Uc], axis=1), f32
        ).astype(NP_BF),
        "w_ax": np.ascontiguousarray(ax, f32).astype(NP_BF),
        "w_am": np.ascontiguousarray(am, f32).astype(NP_BF),
        "w_d1a": np.ascontiguousarray(W1p[0:P], f32),
        "w_d1b": np.ascontiguousarray(W1p[P : 2 * P], f32),
        "w_d2a": np.ascontiguousarray(d_W2[0:P], f32),
        "w_d2b": np.ascontiguousarray(d_W2[P : 2 * P], f32),
        "w_f": np.ascontiguousarray(f_W, f32),
        "biases": biases,
    }

    in_maps = []
    for cidx in range(NCORES):
        sh = inputs_np[cidx * BL : (cidx + 1) * BL]  # [BL, T+1, D]
        hist_t = np.ascontiguousarray(sh[:, :T, :].transpose(1, 2, 0), f32)
        news_t = np.ascontiguousarray(sh[:, T, :].T, f32)  # [D, BL]
        # anews = abc^T @ news  (per-core, hi+lo fp8 pair)
        anews = (abc.T @ news_t).astype(f32)  # [P, BL]
        an_hi = anews.astype(NP_F8)
        an_lo = (anews - an_hi.astype(f32)).astype(NP_F8)
        an_pair = np.zeros((P, 2, BL), NP_F8)
        an_pair[:, 0, :] = an_hi
        an_pair[:, 1, :] = an_lo
        m = dict(shared)
        m["hist8"] = hist_t.astype(NP_F8)
        m["news_b"] = news_t.astype(NP_BF)
        m["news_r"] = news_t
        m["anews8"] = an_pair
        in_maps.append(m)
    return in_maps


_NC_CACHE = {}


def get_nc(debug=False):
    key = (debug,)
    if key not in _NC_CACHE:
        _NC_CACHE[key] = build_nc(debug=debug)
    return _NC_CACHE[key]


def kernel(**inputs):
    inputs = {k: np.asarray(v) for k, v in inputs.items()}
    in_maps = prep_inputs(
        inputs["inputs"], inputs["gru_W"], inputs["gru_U"], inputs["gru_b"],
        inputs["att_W1"], inputs["att_b1"], inputs["att_W2"], inputs["att_b2"],
        inputs["att_W3"], inputs["att_b3"], inputs["au_Wu"], inputs["au_bu"],
        inputs["au_Uu"], inputs["au_Wr"], inputs["au_br"], inputs["au_Ur"],
        inputs["au_Wc"], inputs["au_bc"], inputs["au_Uc"], inputs["bn_gamma"],
        inputs["bn_beta"], inputs["bn_mean"], inputs["bn_var"], inputs["d_W1"],
        inputs["d_b1"], inputs["d_W2"], inputs["d_b2"], inputs["f_W"],
        inputs["f_b"],
    )
    nc = get_nc(debug=CFG["debug"])
    res = run_bass_kernel_spmd(nc, in_maps, list(range(NCORES)))
    y = np.concatenate(
        [res.results[c]["y"].reshape(-1)[:, None] for c in range(NCORES)], axis=0
    ).astype(np.float32)
    return y


# revision 19
# speedup vs baseline: 1.2529x; 1.2021x over previous
"""DIEN kernel for Trainium2 (Bass/Tile), 8-way data-parallel over batch.

Layout: transposed activations [feature (<=128 partitions), batch (free dim)].
Per core: 512 batch rows, T=50 steps. GRU / attention / AUGRU fused in one
skewed loop (ATT 1 step behind GRU, AUGRU SKEW_AU behind), head at the end.

v2 design (mixed precision, engine-balanced):
- x-side projections run as fp8e4 DoubleRow matmuls (0.5 cyc/row): the host
  quantizes hist to fp8, and each gate's bias rides the DR pair's second slot
  against a static ones vector (stationary row-0 = bias row), so gate psums
  arrive bias-included with zero extra ops.
- candidate folds: t1 = (phh+b1h)*r is an STT (no DVE fast mode either way)
  written directly as fp8 into the moving pair next to x8; one DR (I | Wh)
  then computes xh + t1 in a single 107ns matmul.
- recurrent state h/h2/hs stays bf16 so every combine TT gets the DVE 2x
  mode; h-side matmuls are plain bf16 (1 cyc/row, same as f32r).
- attention: relu1/relu2 emit fp8 directly, pa2/pa3 are DR-padded; the
  abc@news term is precomputed on host (hi+lo fp8 pair, folded via DR(I|I)).
- ats broadcast via gpsimd partition_broadcast (no PSUM bank, no PE).
- PSUM banks (8): zr[2] ur[2] cand[1] hside[1] pa1[1] pa2+a3 shared[1].
"""
import sys

sys.path.insert(0, "/opt/trn_rl_repo")

import numpy as np
import ml_dtypes

import concourse.bass as bass
import concourse.mybir as mybir
import concourse.tile as tile
from concourse import bacc
from concourse.bass_utils import run_bass_kernel_spmd

B, T, D, U = 4096, 50, 128, 128
NCORES = 8
BL = B // NCORES  # 512
P = 128
F32 = mybir.dt.float32
F32R = mybir.dt.float32r
BF16 = mybir.dt.bfloat16
F8 = mybir.dt.float8e4
NP_F8 = ml_dtypes.float8_e4m3
NP_BF = ml_dtypes.bfloat16
AF = mybir.ActivationFunctionType
OP = mybir.AluOpType
DR = mybir.MatmulPerfMode.DoubleRow
LEAKY = 0.0003

CFG = {
    "skew_au": 6,
    # engine per op: "v" = vector(DVE), "g" = gpsimd(Pool)
    "eng": {
        "t1": "v", "t2": "g", "mmj": "v",
        "d": "v", "w": "v", "hn": "v",
        "u_": "v", "d2": "v", "m2": "v", "hn2": "v",
        "relu1": "g", "relu2": "g", "patb": "g",
    },
    "debug": False,
}

# bias column indices in the packed [128, 16] bias tensor
B1H, B0H, BU, BR2, BC, B1, B2, B3, DB1A, DB1B, DB2, FB = range(12)


def _eng(nc, key):
    return nc.vector if CFG["eng"][key] == "v" else nc.gpsimd


def build_nc(debug=False):
    nc = bacc.Bacc()
    SKEW_AU = CFG["skew_au"]
    NITER = T + SKEW_AU

    # ---------------- DRAM inputs
    hist8 = nc.dram_tensor("hist8", [T, P, BL], F8, kind="ExternalInput")
    news_b = nc.dram_tensor("news_b", [P, BL], BF16, kind="ExternalInput")
    news_r = nc.dram_tensor("news_r", [P, BL], F32R, kind="ExternalInput")
    # fp8 DR stationary pairs [128, 2, 128]: (A on slot0, B on slot1)
    w_z8 = nc.dram_tensor("w_z8", [P, 2, P], F8, kind="ExternalInput")  # (Wz | e0*bz)
    w_r8 = nc.dram_tensor("w_r8", [P, 2, P], F8, kind="ExternalInput")  # (Wr | e0*br)
    w_c8 = nc.dram_tensor("w_c8", [P, 2, P], F8, kind="ExternalInput")  # (I  | Wh)
    w_vc8 = nc.dram_tensor("w_vc8", [P, 2, P], F8, kind="ExternalInput")  # (I | 0) AUGRU fold
    w_a28 = nc.dram_tensor("w_a28", [P, 2, 64], F8, kind="ExternalInput")  # (a2w | 0)
    w_an8 = nc.dram_tensor("w_an8", [P, 2, P], F8, kind="ExternalInput")  # (I | I) anews fold
    w_a38 = nc.dram_tensor("w_a38", [64, 8, 4], F8, kind="ExternalInput")  # 4x (w3col | 0)
    w_bu8 = nc.dram_tensor("w_bu8", [1, 2, P], F8, kind="ExternalInput")  # (0 | bu row)
    # bf16 weights (h-side + attention + AUGRU x-side)
    w_gu = nc.dram_tensor("w_gu", [P, 3 * U], BF16, kind="ExternalInput")
    w_aw = nc.dram_tensor("w_aw", [P, 3 * U], BF16, kind="ExternalInput")
    w_au = nc.dram_tensor("w_au", [P, 3 * U], BF16, kind="ExternalInput")
    w_ax = nc.dram_tensor("w_ax", [P, P], BF16, kind="ExternalInput")
    w_am = nc.dram_tensor("w_am", [P, P], BF16, kind="ExternalInput")
    anews8 = nc.dram_tensor("anews8", [P, 2, BL], F8, kind="ExternalInput")  # (hi | lo)
    # head (f32r as baseline)
    w_d1a = nc.dram_tensor("w_d1a", [P, 256], F32R, kind="ExternalInput")
    w_d1b = nc.dram_tensor("w_d1b", [P, 256], F32R, kind="ExternalInput")
    w_d2a = nc.dram_tensor("w_d2a", [P, P], F32R, kind="ExternalInput")
    w_d2b = nc.dram_tensor("w_d2b", [P, P], F32R, kind="ExternalInput")
    w_f = nc.dram_tensor("w_f", [P, 1], F32R, kind="ExternalInput")
    biases = nc.dram_tensor("biases", [P, 16], F32, kind="ExternalInput")
    y_out = nc.dram_tensor("y", [1, BL], F32, kind="ExternalOutput")
    if debug:
        hg_out = nc.dram_tensor("hg", [P, BL], F32, kind="ExternalOutput")
        h2_out = nc.dram_tensor("h2f", [P, BL], F32, kind="ExternalOutput")

    with tile.TileContext(nc) as tc:
        import contextlib

        ctx = contextlib.ExitStack()
        with ctx:
            wp = ctx.enter_context(tc.tile_pool(name="wp", bufs=1))
            ps = ctx.enter_context(tc.tile_pool(name="ps", bufs=1, space="PSUM"))

            # ---------------- load weights
            def wtile(name, dram, shape, dt):
                t = wp.tile(shape, dt, name=name, tag=name)
                nc.sync.dma_start(t[:], dram[:])
                return t

            z8 = wtile("z8", w_z8, [P, 2, P], F8)
            r8 = wtile("r8", w_r8, [P, 2, P], F8)
            c8 = wtile("c8", w_c8, [P, 2, P], F8)
            vc8 = wtile("vc8", w_vc8, [P, 2, P], F8)
            a28 = wtile("a28", w_a28, [P, 2, 64], F8)
            an8 = wtile("an8", w_an8, [P, 2, P], F8)
            a38 = wtile("a38", w_a38, [64, 8, 4], F8)
            bu8 = wtile("bu8", w_bu8, [1, 2, P], F8)
            gu = wtile("gu", w_gu, [P, 3 * U], BF16)
            aw = wtile("aw", w_aw, [P, 3 * U], BF16)
            au = wtile("au", w_au, [P, 3 * U], BF16)
            ax = wtile("ax", w_ax, [P, P], BF16)
            am = wtile("am", w_am, [P, P], BF16)
            anp = wtile("anp", anews8, [P, 2, BL], F8)
            d1a = wtile("d1a", w_d1a, [P, 256], F32R)
            d1b = wtile("d1b", w_d1b, [P, 256], F32R)
            d2a = wtile("d2a", w_d2a, [P, P], F32R)
            d2b = wtile("d2b", w_d2b, [P, P], F32R)
            fw = wtile("fw", w_f, [P, 1], F32R)
            bia = wtile("bia", biases, [P, 16], F32)
            newsb = wtile("newsb", news_b, [P, BL], BF16)
            newsr = wtile("newsr", news_r, [P, BL], F32R)

            def bap(col, rows=P):
                return bia[0:rows, col : col + 1]

            # ---------------- static SBUF rings (manual)
            NG = 3
            grings = []  # [t18 | x8 | ones8] fp8
            for k in range(NG):
                g = wp.tile([P, 3, BL], F8, name=f"gring{k}", tag=f"gring{k}")
                nc.vector.memset(g[:, 2, :], 1.0)
                grings.append(g)
            vrings = []  # [t28 | zeros] fp8
            for k in range(2):
                v = wp.tile([P, 2, BL], F8, name=f"vring{k}", tag=f"vring{k}")
                nc.vector.memset(v[:, 1, :], 0.0)
                vrings.append(v)
            a1rings = []  # [a1 | zeros] fp8
            for k in range(2):
                a1r = wp.tile([P, 2, BL], F8, name=f"a1ring{k}", tag=f"a1ring{k}")
                nc.vector.memset(a1r[:, 1, :], 0.0)
                a1rings.append(a1r)
            a2rings = []  # [a2 | zeros] fp8 (64 partitions)
            for k in range(2):
                a2r = wp.tile([64, 2, BL], F8, name=f"a2ring{k}", tag=f"a2ring{k}")
                nc.vector.memset(a2r[:, 1, :], 0.0)
                a2rings.append(a2r)
            onesp = wp.tile([1, 2, BL], F8, name="onesp", tag="onesp")
            nc.vector.memset(onesp[:, 0, :], 0.0)
            nc.vector.memset(onesp[:, 1, :], 1.0)

            NH = 8
            hs_ring = []  # GRU outputs bf16
            for k in range(NH):
                h = wp.tile([P, BL], BF16, name=f"hs{k}", tag=f"hs{k}")
                hs_ring.append(h)
            h2_ring = []
            for k in range(2):
                h2 = wp.tile([P, BL], BF16, name=f"h2_{k}", tag=f"h2_{k}")
                h2_ring.append(h2)
            h_init = wp.tile([P, BL], BF16, name="h_init", tag="h_init")
            nc.vector.memset(h_init[:], 0.0)
            h2_init = wp.tile([P, BL], BF16, name="h2_init", tag="h2_init")
            nc.vector.memset(h2_init[:], 0.0)

            # per-iter small tiles via pools (auto ring by tag)
            sp = ctx.enter_context(tc.tile_pool(name="sp", bufs=2))

            # PSUM banks (8): rr2[2] (r|r2 segments), zu[2] (z|u merged
            # sigmoid), gcand[1] (phh then pxh: pxh's DR already waits the
            # t18 read of phh), acand[1] (prc then pxc, vc8-DR is the start),
            # pa1[1], pa2+a3 partition-split in one persistent bank[1].
            def ps_tile(name, shape, tag, bufs=1):
                return ps.tile(shape, F32, name=name, tag=tag, bufs=bufs)

            mm = nc.tensor.matmul

            hs_at = lambda t: h_init if t < 0 else hs_ring[t % NH]
            h2_at = lambda s: h2_init if s < 0 else h2_ring[s % 2]

            ats_tiles = {}

            pa2a3 = ps.tile([P, BL], F32, name="pa2a3", tag="pa2a3")

            # prefetch x8 for t=0
            nc.sync.dma_start(grings[0][:, 1, :], hist8[0])

            for i in range(NITER):
                t = i if i < T else None
                j = i - 1 if 0 <= i - 1 < T else None
                s = i - SKEW_AU if 0 <= i - SKEW_AU < T else None

                G = grings[t % NG] if t is not None else None
                h_prev = hs_at(t - 1) if t is not None else None
                hs_j = hs_at(j) if j is not None else None
                V = vrings[s % 2] if s is not None else None
                h2_prev = h2_at(s - 1) if s is not None else None
                hs_s = hs_at(s) if s is not None else None

                # ===== phase A: iteration-start-ready matmuls (PE) =====
                if t is not None:
                    if t + 1 < T:
                        nc.sync.dma_start(
                            grings[(t + 1) % NG][:, 1, :], hist8[t + 1]
                        )
                    pr = ps_tile(f"pr{t}", [P, BL], "r")
                    mm(pr[:], r8[:], G[:, 1:3, :], start=True,
                       stop=False, perf_mode=DR)
                    mm(pr[:], gu[:, U : 2 * U], h_prev[:],
                       start=False, stop=True)
                if s is not None:
                    pr2 = ps_tile(f"pr2{s}", [P, BL], "r2")
                    mm(pr2[:], aw[:, U : 2 * U], hs_s[:],
                       start=True, stop=False)
                    mm(pr2[:], au[:, U : 2 * U], h2_prev[:],
                       start=False, stop=True)
                if t is not None:
                    pz = ps_tile(f"pz{t}", [P, BL], "z")
                    mm(pz[:], z8[:], G[:, 1:3, :], start=True,
                       stop=False, perf_mode=DR)
                    mm(pz[:], gu[:, 0:U], h_prev[:], start=False,
                       stop=True)
                if s is not None:
                    pu = ps_tile(f"pu{s}", [P, BL], "u")
                    mm(pu[:], aw[:, 0:U], hs_s[:], start=True,
                       stop=False)
                    mm(pu[:], au[:, 0:U], h2_prev[:],
                       start=False, stop=True)
                if t is not None:
                    phh = ps_tile(f"phh{t}", [P, BL], "gcand")
                    mm(phh[:], gu[:, 2 * U : 3 * U], h_prev[:], start=True,
                       stop=True)
                if s is not None:
                    prc = ps_tile(f"prc{s}", [P, BL], "acand")
                    mm(prc[:], au[:, 2 * U : 3 * U], h2_prev[:], start=True,
                       stop=True)

                # ===== ready elementwise: mmj (DVE), patb (Pool) =====
                if j is not None:
                    mmj = sp.tile([P, BL], BF16, name=f"mmj{j}", tag="mmj")
                    _eng(nc, "mmj").tensor_mul(mmj[:], hs_j[:], newsb[:])
                if s is not None:
                    pat = sp.tile([P, BL], BF16, name=f"pat{s}", tag="pat")
                    nc.gpsimd.partition_broadcast(
                        pat[:], ats_tiles[s // 4][s % 4 : s % 4 + 1, :]
                    )

                # ===== chain heads: sigma-r, sigma-r2 (ACT) =====
                if t is not None:
                    rt = sp.tile([P, BL], BF16, name=f"r{t}", tag="r_sb")
                    nc.scalar.activation(rt[:], pr[:], AF.Sigmoid)
                if s is not None:
                    r2t = sp.tile([P, BL], BF16, name=f"r2{s}", tag="r2_sb")
                    nc.scalar.activation(r2t[:], pr2[:],
                                         AF.Sigmoid, bias=bap(BR2))

                # ===== folds: t18 (DVE), t28 (Pool) =====
                if t is not None:
                    _eng(nc, "t1").scalar_tensor_tensor(
                        G[:, 0, :], phh[:], bap(B1H), rt[:], OP.add, OP.mult
                    )
                if s is not None:
                    _eng(nc, "t2").scalar_tensor_tensor(
                        V[:, 0, :], prc[:], 0.0, r2t[:], OP.add, OP.mult
                    )

                # ===== sigma-z, sigma-u (ACT) =====
                if t is not None:
                    zt = sp.tile([P, BL], BF16, name=f"z{t}", tag="z_sb")
                    nc.scalar.activation(zt[:], pz[:], AF.Sigmoid)
                    z = zt[:]
                if s is not None:
                    ut = sp.tile([P, BL], BF16, name=f"u{s}", tag="u_sb")
                    nc.scalar.activation(ut[:], pu[:], AF.Sigmoid,
                                         bias=bap(BU))

                # ===== attention pa1 (PE) + relu1 (Pool) =====
                if j is not None:
                    pa1 = ps_tile(f"pa1{j}", [P, BL], "pa1")
                    mm(pa1[:], an8[:], anp[:], start=True, stop=False,
                       perf_mode=DR)
                    mm(pa1[:], ax[:], hs_j[:], start=False, stop=False)
                    mm(pa1[:], am[:], mmj[:], start=False, stop=True)
                    A1 = a1rings[j % 2]
                    _eng(nc, "relu1").tensor_scalar(
                        A1[:, 0, :], pa1[:], bap(B1), 0.0, OP.add, OP.max
                    )

                # ===== candidates: pxh-DR + tanh-hc, pxc + tanh-c =====
                if t is not None:
                    pxh = ps_tile(f"pxh{t}", [P, BL], "gcand")
                    mm(pxh[:], c8[:], G[:, 0:2, :], start=True, stop=True,
                       perf_mode=DR)
                    hc = sp.tile([P, BL], BF16, name=f"hc{t}", tag="hc")
                    nc.scalar.activation(hc[:], pxh[:], AF.Tanh, bias=bap(B0H))
                if s is not None:
                    pxc = ps_tile(f"pxc{s}", [P, BL], "acand")
                    mm(pxc[:], vc8[:], V[:, 0:2, :], start=True, stop=False,
                       perf_mode=DR)
                    mm(pxc[:], aw[:, 2 * U : 3 * U], hs_s[:], start=False,
                       stop=True)
                    c = sp.tile([P, BL], BF16, name=f"c{s}", tag="c")
                    nc.scalar.activation(c[:], pxc[:], AF.Tanh, bias=bap(BC))

                # ===== combines (DVE): GRU first, then AUGRU =====
                if t is not None:
                    # hn = hc + z*(h - hc)
                    d = sp.tile([P, BL], BF16, name=f"d{t}", tag="d")
                    _eng(nc, "d").tensor_sub(d[:], h_prev[:], hc[:])
                    w_ = sp.tile([P, BL], BF16, name=f"w{t}", tag="w")
                    _eng(nc, "w").tensor_mul(w_[:], z, d[:])
                    hn = hs_ring[t % NH]
                    _eng(nc, "hn").tensor_add(hn[:], hc[:], w_[:])
                if s is not None:
                    # hn2 = h2 + u_*(c - h2), u_ = pat*u
                    u_ = sp.tile([P, BL], BF16, name=f"u_{s}", tag="u_")
                    _eng(nc, "u_").tensor_mul(u_[:], pat[:], ut[:])
                    d2 = sp.tile([P, BL], BF16, name=f"d2{s}", tag="d2")
                    _eng(nc, "d2").tensor_sub(d2[:], c[:], h2_prev[:])
                    m2 = sp.tile([P, BL], BF16, name=f"m2{s}", tag="m2")
                    _eng(nc, "m2").tensor_mul(m2[:], u_[:], d2[:])
                    hn2 = h2_ring[s % 2]
                    _eng(nc, "hn2").tensor_add(hn2[:], h2_prev[:], m2[:])

                # ===== attention tail: pa2, relu2, pa3, [ats sigma] =====
                if j is not None:
                    mm(pa2a3[0:64, :], a28[:], A1[:, 0:2, :], start=True,
                       stop=True, perf_mode=DR)
                    A2 = a2rings[j % 2]
                    _eng(nc, "relu2").tensor_scalar(
                        A2[:, 0, :], pa2a3[0:64, :], bap(B2, rows=64), 0.0,
                        OP.add, OP.max
                    )
                    g = j // 4
                    k4 = j % 4
                    mm(pa2a3[64:68, :], a38[:, 2 * k4 : 2 * k4 + 2, :],
                       A2[:, 0:2, :], start=(k4 == 0),
                       stop=(k4 == 3 or j == T - 1), perf_mode=DR)
                    if k4 == 3 or j == T - 1:
                        k = k4 + 1
                        ats = sp.tile([4, BL], BF16, name=f"ats{g}", tag="ats")
                        nc.scalar.activation(
                            ats[0:k, :], pa2a3[64 : 64 + k, :], AF.Sigmoid,
                            bias=bap(B3, rows=k),
                        )
                        ats_tiles[g] = ats



            # ---------------- deep head (one-time, f32r)
            h2f = h2_ring[(T - 1) % 2]
            if debug:
                hgf = sp.tile([P, BL], F32, name="hgf", tag="hgf")
                nc.scalar.copy(hgf[:], hs_ring[(T - 1) % NH][:])
                nc.sync.dma_start(hg_out[:], hgf[:])
                h2c = sp.tile([P, BL], F32, name="h2c", tag="h2c")
                nc.scalar.copy(h2c[:], h2f[:])
                nc.sync.dma_start(h2_out[:], h2c[:])

            o1 = sp.tile([P, 2 * BL], F32R, name="o1", tag="o1")
            for mch in range(2):
                po = ps_tile(f"po1_{mch}", [P, BL], "gcand")
                mm(po[:], d1a[:, mch * P : (mch + 1) * P], h2f[:], start=True,
                   stop=False)
                mm(po[:], d1b[:, mch * P : (mch + 1) * P], newsr[:], start=False,
                   stop=True)
                nc.scalar.activation(
                    o1[:, mch * BL : (mch + 1) * BL], po[:], AF.Lrelu,
                    bias=bap(DB1A + mch), alpha=LEAKY,
                )
            po2 = ps_tile("po2", [P, BL], "acand")
            mm(po2[:], d2a[:], o1[:, 0:BL], start=True, stop=False)
            mm(po2[:], d2b[:], o1[:, BL : 2 * BL], start=False, stop=True)
            o2 = sp.tile([P, BL], F32R, name="o2", tag="o2")
            nc.scalar.activation(o2[:], po2[:], AF.Lrelu, bias=bap(DB2),
                                 alpha=LEAKY)
            py = ps_tile("py", [1, BL], "pa1")
            mm(py[:], fw[:], o2[:], start=True, stop=True)
            y_sb = sp.tile([1, BL], F32, name="y_sb", tag="ysb")
            nc.scalar.activation(y_sb[:], py[:], AF.Sigmoid, bias=bap(FB, rows=1))
            nc.sync.dma_start(y_out[:], y_sb[:])

    nc.compile()
    return nc


def prep_inputs(inputs_np, gru_W, gru_U, gru_b, att_W1, att_b1, att_W2, att_b2,
                att_W3, att_b3, au_Wu, au_bu, au_Uu, au_Wr, au_br, au_Ur,
                au_Wc, au_bc, au_Uc, bn_gamma, bn_beta, bn_mean, bn_var,
                d_W1, d_b1, d_W2, d_b2, f_W, f_b):
    """Host-side preprocessing. Returns per-core input maps."""
    f32 = np.float32

    biases = np.zeros((P, 16), f32)
    biases[:, B1H] = gru_b[1, 2 * U : 3 * U]
    biases[:, B0H] = gru_b[0, 2 * U : 3 * U]
    biases[:, BU] = au_bu
    biases[:, BR2] = au_br
    biases[:, BC] = au_bc
    biases[:, B1] = att_b1
    biases[0:64, B2] = att_b2
    biases[0:4, B3] = att_b3[0]

    def dr_pair(a, b):
        out = np.zeros((P, 2, a.shape[1]), NP_F8)
        out[:, 0, :] = a.astype(NP_F8)
        out[:, 1, :] = b.astype(NP_F8)
        return out

    bz = gru_b[0, 0:U] + gru_b[1, 0:U]
    br = gru_b[0, U : 2 * U] + gru_b[1, U : 2 * U]
    rider_z = np.zeros((P, P), f32)
    rider_z[0, :] = bz
    rider_r = np.zeros((P, P), f32)
    rider_r[0, :] = br
    ident = np.eye(P, dtype=f32)

    ax = att_W1[0:P] - att_W1[P : 2 * P]
    am = att_W1[3 * P : 4 * P]
    abc = att_W1[P : 2 * P] + att_W1[2 * P : 3 * P]

    a2pair = np.zeros((P, 2, 64), NP_F8)
    a2pair[:, 0, :] = att_W2.astype(NP_F8)

    # pa3: per k4, (w3 one-hot column k4 | 0) as [64, 2, 4] blocks -> [64, 8, 4]
    a3pair = np.zeros((64, 8, 4), NP_F8)
    for k in range(4):
        col = np.zeros((64, 4), f32)
        col[:, k] = att_W3[:, 0]
        a3pair[:, 2 * k, :] = col.astype(NP_F8)

    bu8 = np.zeros((1, 2, P), NP_F8)
    bu8[0, 1, :] = au_bu.astype(NP_F8)

    # BN folded into head layer 1
    s = (bn_gamma / np.sqrt(bn_var + 1e-3)).astype(f32)
    tt = (bn_beta - bn_mean * s).astype(f32)
    W1p = (s[:, None] * d_W1).astype(f32)
    b1p = (tt @ d_W1 + d_b1).astype(f32)
    biases[:, DB1A] = b1p[0:P]
    biases[:, DB1B] = b1p[P : 2 * P]
    biases[:, DB2] = d_b2
    biases[0, FB] = f_b[0]

    shared = {
        "w_z8": dr_pair(gru_W[:, 0:U], rider_z),
        "w_r8": dr_pair(gru_W[:, U : 2 * U], rider_r),
        "w_c8": dr_pair(ident, gru_W[:, 2 * U : 3 * U]),
        "w_vc8": dr_pair(ident, np.zeros((P, P), f32)),
        "w_a28": a2pair,
        "w_an8": dr_pair(ident, ident),
        "w_a38": a3pair,
        "w_bu8": bu8,
        "w_gu": np.ascontiguousarray(gru_U, f32).astype(NP_BF),
        "w_aw": np.ascontiguousarray(
            np.concatenate([au_Wu, au_Wr, au_Wc], axis=1), f32
        ).astype(NP_BF),
        "w_au": np.ascontiguousarray(
            np.concatenate([au_Uu, au_Ur, au_Uc], axis=1), f32
        ).astype(NP_BF),
        "w_ax": np.ascontiguousarray(ax, f32).astype(NP_BF),
        "w_am": np.ascontiguousarray(am, f32).astype(NP_BF),
        "w_d1a": np.ascontiguousarray(W1p[0:P], f32),
        "w_d1b": np.ascontiguousarray(W1p[P : 2 * P], f32),
        "w_d2a": np.ascontiguousarray(d_W2[0:P], f32),
        "w_d2b": np.ascontiguousarray(d_W2[P : 2 * P], f32),
        "w_f": np.ascontiguousarray(f_W, f32),
        "biases": biases,
    }

    in_maps = []
    for cidx in range(NCORES):
        sh = inputs_np[cidx * BL : (cidx + 1) * BL]  # [BL, T+1, D]
        hist_t = np.ascontiguousarray(sh[:, :T, :].transpose(1, 2, 0), f32)
        news_t = np.ascontiguousarray(sh[:, T, :].T, f32)  # [D, BL]
        # anews = abc^T @ news  (per-core, hi+lo fp8 pair)
        anews = (abc.T @ news_t).astype(f32)  # [P, BL]
        an_hi = anews.astype(NP_F8)
        an_lo = (anews - an_hi.astype(f32)).astype(NP_F8)
        an_pair = np.zeros((P, 2, BL), NP_F8)
        an_pair[:, 0, :] = an_hi
        an_pair[:, 1, :] = an_lo
        m = dict(shared)
        m["hist8"] = hist_t.astype(NP_F8)
        m["news_b"] = news_t.astype(NP_BF)
        m["news_r"] = news_t
        m["anews8"] = an_pair
        in_maps.append(m)
    return in_maps


_NC_CACHE = {}


def get_nc(debug=False):
    key = (debug,)
    if key not in _NC_CACHE:
        _NC_CACHE[key] = build_nc(debug=debug)
    return _NC_CACHE[key]


def kernel(**inputs):
    inputs = {k: np.asarray(v) for k, v in inputs.items()}
    in_maps = prep_inputs(
        inputs["inputs"], inputs["gru_W"], inputs["gru_U"], inputs["gru_b"],
        inputs["att_W1"], inputs["att_b1"], inputs["att_W2"], inputs["att_b2"],
        inputs["att_W3"], inputs["att_b3"], inputs["au_Wu"], inputs["au_bu"],
        inputs["au_Uu"], inputs["au_Wr"], inputs["au_br"], inputs["au_Ur"],
        inputs["au_Wc"], inputs["au_bc"], inputs["au_Uc"], inputs["bn_gamma"],
        inputs["bn_beta"], inputs["bn_mean"], inputs["bn_var"], inputs["d_W1"],
        inputs["d_b1"], inputs["d_W2"], inputs["d_b2"], inputs["f_W"],
        inputs["f_b"],
    )
    nc = get_nc(debug=CFG["debug"])
    res = run_bass_kernel_spmd(nc, in_maps, list(range(NCORES)))
    y = np.concatenate(
        [res.results[c]["y"].reshape(-1)[:, None] for c in range(NCORES)], axis=0
    ).astype(np.float32)
    return y
